# revision 22
# baseline (speedup 1.0000x reference)
"""Trainium2 Bass kernel for the dense CNN (CSP block with CARAFE upsamplers).

Strategy: pure data parallelism — 8 samples over 8 NeuronCores, one sample per
core, full forward pass per core:

  cv1 (1x1, 256->256) -> split y / y0
  bneck1 = two 3x3 CBS convs (128->128)        [y1]
  cvm2 (1x1 over pixel_unshuffle(y1,2))        [z2, 64x64]
  carafe(k=1,up=2) == nearest-neighbor 2x upsample (softmax over a single
      channel is identically 1 -> its down/enc convs are dead code) + y1
  bneck2                                        [y2]
  cvm3 (1x1 over pixel_unshuffle(y2,4))        [z3, 32x32]
  carafe(k=2,up=4): per-pixel 2x2-tap softmax weights, 4x up + y2  [y3pre]
  bneck3                                        [y3]
  cv2 (1x1, 640->256) over concat(y, y0, y1, y2, y3)

Implementation notes:
- Feature maps live in SBUF as [128 ch, 130*130] bf16 with a zero halo; a 3x3
  conv is 9 shifted-tap matmuls accumulated in fp32 PSUM (bf16 streams at the
  PE's full 1 elem/cell/cycle rate); each conv epilogue is one ScalarE op:
  silu(psum*scale + bias) with per-partition f32 scale/bias.
- pixel_unshuffle feeds the 1x1 convs as strided-AP k-slices (host-side
  weight re-layout), so no data movement is spent on it.
- CARAFE k=2/up=4: softmax weights computed on-chip ([64,1024], exp on
  ScalarE, tap-sum via a mod-16 indicator matmul), bounced to DRAM, and
  partition-broadcast back per (row-half, subpos-row) chunk via SWDGE DMA.
  Since the weights sum to 1, the 4-tap blend is z00 + sum w_t*(z_t - z00)
  (3 multiplies instead of 4) on the DVE at the bf16 2x rate, with an even
  (34) row stride + a column-shifted copy of z3 to keep reads 4B-aligned.
- cv2 (640->256, 1x1) is split: k-tiles {y,y0,y1,y2} are computed as a bf16
  partial (acc4, bounced through DRAM) on the PE during the CARAFE DVE window;
  the y3 k-tile + an identity-matmul re-injection of acc4 are chained per
  4-row tile directly behind bneck3's second conv, so PE never waits on a
  whole-map dependency at the end.
- y3pre is built as two half-maps (rows -1..64 / 63..128) so bneck3's first
  conv starts while the second half of the reassembly is still on the DVE.
- y/y0/y1 spill to DRAM (bf16) and stream back for the cv2 partial; all
  weights are pre-transposed/pre-sliced and pre-cast to bf16 on the host.

Next identified (unshipped) optimization: split y2 into two halo-free
half-maps (rows 0..63 / 64..127). No 3x3 conv reads y2 -- only cvm3,
the cv2 partial, and the CARAFE residual, all of which read within one
half -- so the split needs no boundary duplication, and cvm3's first
tile (+ the whole kernel-prediction chain gating the CARAFE DVE work)
could start ~25us earlier, during bneck2's second half. Beyond that,
the remaining ~55us over the matmul roofline is: the fixed ~6us drain
barrier, ~6us startup DMA latency, small-N matmul inefficiency in
cvm2/cvm3/kt, and boundary stalls whose measured fixes all regressed.
fp8+DoubleRow on the six 3x3 convs is the only large remaining lever
(~2x PE peak) and requires an accuracy-gate revalidation.
"""
import sys

sys.path.insert(0, "/opt/trn_rl_repo")

import numpy as np
import ml_dtypes

import concourse.bass as bass
import concourse.bacc as bacc
import concourse.mybir as mybir
import concourse.tile as tile

F32 = mybir.dt.float32
BF16 = mybir.dt.bfloat16
F8 = mybir.dt.float8e4
AF = mybir.ActivationFunctionType
DR = mybir.MatmulPerfMode.DoubleRow
WS = 64.0  # fp8 3x3-conv weight pre-scale (folded out via the BN scale)

N_CORES = 8
C = 128
H = W = 128
HP = H + 2  # padded row length
NPIX = H * W
IOFF = HP + 1  # offset of interior (1,1) in padded layout
ZP = 33  # z3 logical padded side (rows/cols -1..31)
ZS = 34  # z3 row stride (even, for DVE 2x alignment)
TAPS9 = [(dy, dx) for dy in (-1, 0, 1) for dx in (-1, 0, 1)]


def _ap(t, off, dims):
    """Free-dim AP into tile t (keeps full partition dim)."""
    return bass.AP(tensor=t.tensor, offset=t.offset + off, ap=[list(t.ap[0])] + dims)


def _zero_border(nc, mp, side=HP):
    nc.vector.memset(_ap(mp, 0, [[1, side]]), 0.0)
    nc.vector.memset(_ap(mp, (side - 1) * side, [[1, side]]), 0.0)
    nc.vector.memset(_ap(mp, side, [[side, side - 2]]), 0.0)
    nc.vector.memset(_ap(mp, side + side - 1, [[side, side - 2]]), 0.0)


def build_nc():
    nc = bacc.Bacc(None)

    # ---- I/O ----
    x_d = nc.dram_tensor("x", [2, C, NPIX], BF16, kind="ExternalInput")
    w_m = {}
    for name in ("m1a", "m1b", "m2a", "m2b", "m3a", "m3b"):
        w_m[name] = nc.dram_tensor(f"w_{name}", [9, C, C], F8, kind="ExternalInput")
    w_cv1 = nc.dram_tensor("w_cv1", [2, C, 2 * C], BF16, kind="ExternalInput")
    w_cvm2 = nc.dram_tensor("w_cvm2", [4, C, C], BF16, kind="ExternalInput")
    w_cvm3 = nc.dram_tensor("w_cvm3", [16, C, C], BF16, kind="ExternalInput")
    w_cv2 = nc.dram_tensor("w_cv2", [5, C, 2 * C], BF16, kind="ExternalInput")
    w_dn = nc.dram_tensor("w_dn", [C, 32], BF16, kind="ExternalInput")
    w_en = nc.dram_tensor("w_en", [4, 32, 64], BF16, kind="ExternalInput")
    s64_d = nc.dram_tensor("s64", [64, 64], F32, kind="ExternalInput")
    id_d = nc.dram_tensor("ident", [C, C], BF16, kind="ExternalInput")
    sball_d = nc.dram_tensor("sball", [C, 26], F32, kind="ExternalInput")
    out_d = nc.dram_tensor("out", [2, C, NPIX], F32, kind="ExternalOutput")

    # ---- DRAM scratch ----
    y_sp = nc.dram_tensor("y_sp", [C, NPIX], BF16)  # px-linear
    pad_sp = {k: nc.dram_tensor(f"{k}_sp", [C, HP * HP], BF16) for k in ("y0", "y1")}
    wn_sp = nc.dram_tensor("wn_sp", [64 * 1024], BF16)
    acc4_sp = nc.dram_tensor("acc4_sp", [2, C, NPIX], BF16)  # cv2 partial (y,y0,y1,y2)

    with tile.TileContext(nc) as tc:
        import contextlib

        est = contextlib.ExitStack()
        with est:
            consts = est.enter_context(tc.tile_pool(name="consts", bufs=1))
            maps = est.enter_context(tc.tile_pool(name="maps", bufs=2))
            psum = est.enter_context(tc.tile_pool(name="psum", bufs=4, space="PSUM"))
            # cv2-partial stream tiles (y0,y1 padded row-blocks); opened at top
            # level so stages 0-3 can prefetch during bneck2's PE-bound window
            phFpre = est.enter_context(tc.tile_pool(name="phFpre", bufs=3))
            cts = {}

            def load_ct(st):
                ct = phFpre.tile([C, 2 * 2080], BF16, tag="ct4", name="ct4")
                poff = (st * 16 + 1) * HP
                nc.sync.dma_start(out=ct[:, 0:2080],
                                  in_=pad_sp["y0"][:, poff:poff + 2080])
                nc.sync.dma_start(out=ct[:, 2080:2 * 2080],
                                  in_=pad_sp["y1"][:, poff:poff + 2080])
                cts[st] = ct

            # ---- constants (small, plus cv2 weights + identity: persistent) ----
            sball = consts.tile([C, 26], F32, tag="sball", name="sball")
            nc.sync.dma_start(out=sball, in_=sball_d[:, :])
            sb = {}
            off = 0
            for name, w_ in (("cv1", 4), ("cv2", 4), ("m1a", 2), ("m1b", 2),
                             ("m2a", 2), ("m2b", 2), ("m3a", 2), ("m3b", 2),
                             ("cvm2", 2), ("cvm3", 2)):
                sb[name] = sball[:, off:off + w_]
                off += w_
            sb["dn_b"] = sball[0:32, 24:25]
            sb["en_b"] = sball[0:64, 25:26]

            s64 = consts.tile([64, 64], F32, tag="s64")
            nc.sync.dma_start(out=s64, in_=s64_d[:, :])
            wdn = consts.tile([C, 32], BF16, tag="wdn")
            nc.sync.dma_start(out=wdn, in_=w_dn[:, :])
            wen = consts.tile([32, 4, 64], BF16, tag="wen")
            nc.sync.dma_start(out=wen, in_=w_en[:, :, :].rearrange("t p n -> p t n"))
            ident = consts.tile([C, C], BF16, tag="ident")
            nc.sync.dma_start(out=ident, in_=id_d[:, :])
            wcv2 = consts.tile([C, 5, 2 * C], BF16, tag="wcv2")
            nc.sync.dma_start(out=wcv2, in_=w_cv2[:, :, :].rearrange("t p n -> p t n"))
            wsb = {}

            def load_w(pool, name, src, shape, dt=BF16):
                t = pool.tile(shape, dt, tag=f"w_{name}", name=f"w_{name}")
                nc.sync.dma_start(out=t, in_=src[:, :, :].rearrange("t p n -> p t n"))
                wsb[name] = t

            def taps9_fp8(ps, wt, src, off0):
                """9-tap accumulation as 4 fp8 DoubleRow pair-matmuls + 1
                single; off0 = offset of the (dy=-1,dx=-1) tap window."""
                for p in range(4):
                    dy0, dx0 = TAPS9[2 * p]
                    dy1, dx1 = TAPS9[2 * p + 1]
                    delta = (dy1 - dy0) * HP + (dx1 - dx0)
                    rhs = bass.AP(
                        tensor=src.tensor,
                        offset=src.offset + off0 + (dy0 + 1) * HP + (dx0 + 1),
                        ap=[list(src.ap[0]), [delta, 2], [HP, 4], [1, W]],
                    )
                    nc.tensor.matmul(ps, wt[:, 2 * p:2 * p + 2, :], rhs,
                                     start=(p == 0), stop=False, perf_mode=DR)
                rhs = bass.AP(tensor=src.tensor, offset=src.offset + off0 + 2 * HP + 2,
                              ap=[list(src.ap[0]), [HP, 4], [1, W]])
                nc.tensor.matmul(ps, wt[:, 8, :], rhs, start=False, stop=True)

            def conv3x3_split(srcA, srcB, dst, wname):
                """CBS 3x3 whose input lives in two row-half maps (66 rows each:
                A = hi-res rows -1..64, B = rows 63..128)."""
                wt = wsb[wname]
                s_ap, b_ap = sb[wname][:, 0:1], sb[wname][:, 1:2]
                _zero_border(nc, dst)
                for i in range(32):
                    srcm = srcA if i < 16 else srcB
                    ii = i if i < 16 else i - 16
                    ps = psum.tile([C, 512], F32, tag="ps", name="ps")
                    taps9_fp8(ps, wt, srcm, 4 * ii * HP)
                    nc.scalar.activation(
                        _ap(dst, IOFF + 4 * i * HP, [[HP, 4], [1, W]]),
                        ps[:, :].rearrange("p (r w) -> p r w", r=4),
                        AF.Silu, bias=b_ap, scale=s_ap,
                    )

            def conv3x3(src, dst, wname):
                """CBS 3x3: src padded map -> dst padded map (interior)."""
                wt = wsb[wname]
                s_ap, b_ap = sb[wname][:, 0:1], sb[wname][:, 1:2]
                _zero_border(nc, dst)
                for i in range(32):
                    ps = psum.tile([C, 512], F32, tag="ps", name="ps")
                    taps9_fp8(ps, wt, src, 4 * i * HP)
                    nc.scalar.activation(
                        _ap(dst, IOFF + 4 * i * HP, [[HP, 4], [1, W]]),
                        ps[:, :].rearrange("p (r w) -> p r w", r=4),
                        AF.Silu, bias=b_ap, scale=s_ap,
                    )

            with tc.tile_pool(name="wearly", bufs=1) as wearly:
                load_w(wearly, "cv1", w_cv1, [C, 2, 2 * C])

                # ================= Phase A: cv1 =================
                y0p = maps.tile([C, HP * HP], BF16, tag="m")
                _zero_border(nc, y0p)
                # fp8 shadow of y0 for bneck1 (cv2/spill keep the bf16 map);
                # DVE copies chase cv1's stages in the otherwise-idle window
                y0p8 = wearly.tile([C, HP * HP], F8, tag="y08")
                _zero_border(nc, y0p8)
                with tc.tile_pool(name="phA", bufs=3) as phA, \
                     tc.tile_pool(name="phAy", bufs=3) as phAy:
                    for st in range(8):  # stages of 2048 px (16 rows)
                        xt = phA.tile([C, 2, 2048], BF16, tag="xt", name="xt")
                        for kt in range(2):
                            nc.sync.dma_start(
                                out=xt[:, kt, :], in_=x_d[kt, :, st * 2048:(st + 1) * 2048]
                            )
                        yt = phAy.tile([C, 2048], BF16, tag="yt", name="yt")
                        for j in range(4):
                            row0 = 16 * st + 4 * j
                            psy = psum.tile([C, 512], F32, tag="ps", name="ps")
                            psy0 = psum.tile([C, 512], F32, tag="ps", name="ps")
                            for kt in range(2):
                                nc.tensor.matmul(
                                    psy, wsb["cv1"][:, kt, 0:C],
                                    xt[:, kt, j * 512:(j + 1) * 512],
                                    start=(kt == 0), stop=(kt == 1),
                                )
                            for kt in range(2):
                                nc.tensor.matmul(
                                    psy0, wsb["cv1"][:, kt, C:2 * C],
                                    xt[:, kt, j * 512:(j + 1) * 512],
                                    start=(kt == 0), stop=(kt == 1),
                                )
                            nc.scalar.activation(
                                yt[:, j * 512:(j + 1) * 512], psy, AF.Silu,
                                bias=sb["cv1"][:, 2:3], scale=sb["cv1"][:, 0:1],
                            )
                            nc.scalar.activation(
                                _ap(y0p, IOFF + row0 * HP, [[HP, 4], [1, W]]),
                                psy0[:, :].rearrange("p (r w) -> p r w", r=4),
                                AF.Silu, bias=sb["cv1"][:, 3:4], scale=sb["cv1"][:, 1:2],
                            )
                        nc.sync.dma_start(
                            out=y_sp[:, st * 2048:(st + 1) * 2048], in_=yt
                        )
                        nc.vector.tensor_copy(
                            _ap(y0p8, IOFF + 16 * st * HP, [[HP, 16], [1, W]]),
                            _ap(y0p, IOFF + 16 * st * HP, [[HP, 16], [1, W]]),
                        )
                nc.sync.dma_start(out=pad_sp["y0"][:, :], in_=y0p)

                # remaining early weights (loads overlap with phase A/B compute)
                load_w(wearly, "m1a", w_m["m1a"], [C, 9, C], dt=F8)
                load_w(wearly, "m1b", w_m["m1b"], [C, 9, C], dt=F8)
                load_w(wearly, "cvm2", w_cvm2, [C, 4, C])
                load_w(wearly, "m2a", w_m["m2a"], [C, 9, C], dt=F8)
                load_w(wearly, "m2b", w_m["m2b"], [C, 9, C], dt=F8)

                # ================= Phase B: bneck1 =================
                with tc.tile_pool(name="phB", bufs=1) as phB:
                    t1p = phB.tile([C, HP * HP], F8, tag="aux", name="t1p")
                    conv3x3(y0p8, t1p, "m1a")
                    y1p = maps.tile([C, HP * HP], BF16, tag="m")
                    conv3x3(t1p, y1p, "m1b")
                nc.sync.dma_start(out=pad_sp["y1"][:, :], in_=y1p)
                load_w(consts, "cvm3", w_cvm3, [C, 16, C])

                # ================= Phase C/D: cvm2 + carafe2 =================
                with tc.tile_pool(name="phC", bufs=1) as phC:
                    z2h = []
                    y2pp = maps.tile([C, HP * HP], F8, tag="m")
                    _zero_border(nc, y2pp)
                    for half in range(2):
                        z2 = phC.tile([C, 2048], BF16, tag=f"z2{half}", name="z2")
                        z2h.append(z2)
                        for i in range(4):  # 4 rows of 64 px each per psum tile
                            ps = psum.tile([C, 512], F32, tag="ps", name="ps")
                            for abi in range(4):
                                a, b = abi // 2, abi % 2
                                rhs = _ap(y1p, ((64 * half + 16 * i) + a + 1) * HP + (b + 1),
                                          [[2 * HP, 8], [2, 64]])
                                nc.tensor.matmul(ps, wsb["cvm2"][:, abi, :], rhs,
                                                 start=(abi == 0), stop=(abi == 3))
                            nc.scalar.activation(
                                z2[:, i * 512:(i + 1) * 512], ps, AF.Silu,
                                bias=sb["cvm2"][:, 1:2], scale=sb["cvm2"][:, 0:1],
                            )
                        # carafe2 == NN 2x upsample + y1, for this half's rows
                        for abi in range(4):
                            a, b = abi // 2, abi % 2
                            pos = [[2 * HP, 32], [2, 64]]
                            nc.vector.tensor_tensor(
                                out=_ap(y2pp, IOFF + (64 * half + a) * HP + b, pos),
                                in0=z2[:, :].rearrange("p (h w) -> p h w", h=32),
                                in1=_ap(y1p, IOFF + (64 * half + a) * HP + b, pos),
                                op=mybir.AluOpType.add,
                            )

                # ================= Phase E: bneck2 =================
                with tc.tile_pool(name="phE", bufs=1) as phE:
                    t2p = phE.tile([C, HP * HP], F8, tag="aux", name="t2p")
                    conv3x3(y2pp, t2p, "m2a")
                    for st in range(2):
                        load_ct(st)
                    y2p = maps.tile([C, HP * HP], BF16, tag="m")
                    conv3x3(t2p, y2p, "m2b")
                    for st in range(2, 4):
                        load_ct(st)

            # late weights (cvm3 was preloaded into consts during phase B/C)
            with tc.tile_pool(name="wlate", bufs=1) as wlate, \
                 tc.tile_pool(name="phFy3", bufs=1) as phFy3:
                load_w(wlate, "m3a", w_m["m3a"], [C, 9, C], dt=F8)
                load_w(wlate, "m3b", w_m["m3b"], [C, 9, C], dt=F8)

                # ======== Phase F: cvm3 + carafe4 + cv2 partial(y1,y2) ========
                with tc.tile_pool(name="phF", bufs=1) as phF, \
                     tc.tile_pool(name="phFwb", bufs=2) as phFwb, \
                     tc.tile_pool(name="phFtmp", bufs=1) as phFtmp, \
                     tc.tile_pool(name="phFacc", bufs=2) as phFacc, \
                     tc.tile_pool(name="phFy", bufs=2) as phFy, \
                     tc.tile_pool(name="phFac", bufs=2) as phFac, \
                     tc.tile_pool(name="psF", bufs=2, space="PSUM") as psF:
                    # z3, padded top/left, row stride 34 (even: DVE 2x alignment)
                    z3a = phF.tile([C, ZS * ZP], BF16, tag="z3a", name="z3a")
                    nc.vector.memset(z3a, 0.0)
                    for i in range(2):  # 16 z3-rows of 32 px per tile
                        ps = psum.tile([C, 512], F32, tag="ps", name="ps")
                        for abi in range(16):
                            a, b = abi // 4, abi % 4
                            rhs = _ap(y2p, (64 * i + a + 1) * HP + (b + 1),
                                      [[4 * HP, 16], [4, 32]])
                            nc.tensor.matmul(ps, wsb["cvm3"][:, abi, :], rhs,
                                             start=(abi == 0), stop=(abi == 15))
                        nc.scalar.activation(
                            _ap(z3a, (16 * i + 1) * ZS + 1, [[ZS, 16], [1, 32]]),
                            ps[:, :].rearrange("p (r w) -> p r w", r=16),
                            AF.Silu, bias=sb["cvm3"][:, 1:2], scale=sb["cvm3"][:, 0:1],
                        )
                    # column-shifted copy: z3b[r, k] = z3a[r, k+1] (even tap starts)
                    z3b = phF.tile([C, ZS * ZP], BF16, tag="z3b", name="z3b")
                    nc.vector.memset(z3b, 0.0)
                    nc.vector.tensor_copy(
                        _ap(z3b, 0, [[ZS, ZP], [1, ZP - 1]]),
                        _ap(z3a, 1, [[ZS, ZP], [1, ZP - 1]]),
                    )

                    # kernel prediction: down 1x1 (128->32), pad(1,0), enc 2x2 (32->64)
                    kt1p = phF.tile([32, ZP * ZP], BF16, tag="kt1p", name="kt1p")
                    _zero_border(nc, kt1p, side=ZP)
                    for i in range(2):
                        ps1 = psF.tile([32, 512], F32, tag="psk", name="psk")
                        nc.tensor.matmul(
                            ps1, wdn, _ap(z3a, (16 * i + 1) * ZS + 1, [[ZS, 16], [1, 32]]),
                            start=True, stop=True,
                        )
                        nc.scalar.activation(
                            _ap(kt1p, (16 * i + 1) * ZP + 1, [[ZP, 16], [1, 32]]),
                            ps1[:, :].rearrange("p (r w) -> p r w", r=16),
                            AF.Identity, bias=sb["dn_b"][:, 0:1],
                        )
                    e_sb = phF.tile([64, 1024], F32, tag="e", name="e_sb")
                    for i in range(2):
                        ps2 = psF.tile([64, 512], F32, tag="psk", name="psk")
                        for t in range(4):
                            di, dj = t // 2, t % 2
                            rhs = bass.AP(
                                tensor=kt1p.tensor,
                                offset=kt1p.offset + (16 * i + di) * ZP + dj,
                                ap=[list(kt1p.ap[0]), [ZP, 16], [1, 32]],
                            )
                            nc.tensor.matmul(ps2, wen[:, t, :], rhs,
                                             start=(t == 0), stop=(t == 3))
                        nc.scalar.activation(
                            e_sb[:, i * 512:(i + 1) * 512],
                            ps2[:, :].rearrange("p (r w) -> p r w", r=16),
                            AF.Exp, bias=sb["en_b"][:, 0:1],
                        )
                    sm = psF.tile([64, 1024], F32, tag="psk", name="psk")
                    for i in range(2):
                        nc.tensor.matmul(sm[:, i * 512:(i + 1) * 512], s64,
                                         e_sb[:, i * 512:(i + 1) * 512],
                                         start=True, stop=True)
                    rden = phF.tile([64, 1024], F32, tag="rden", name="rden")
                    nc.vector.reciprocal(rden, sm)
                    wnt = phF.tile([64, 1024], BF16, tag="wnt", name="wnt")
                    nc.vector.tensor_tensor(out=wnt, in0=e_sb, in1=rden,
                                            op=mybir.AluOpType.mult)
                    nc.sync.dma_start(
                        out=wn_sp[:].rearrange("(p f) -> p f", p=64), in_=wnt)

                    # ---- cv2 partial: acc4 = w_y.y + w_y0.y0 + w_y1.y1 + w_y2.y2
                    # (emitted before the reassembly so these matmuls fill the PE
                    #  while the DVE does the CARAFE products)
                    for st in range(8):
                        if st >= 4:
                            load_ct(st)
                        ct = cts.pop(st)
                        yseg = phFy.tile([C, 2048], BF16, tag="yseg", name="yseg")
                        nc.sync.dma_start(out=yseg,
                                          in_=y_sp[:, st * 2048:(st + 1) * 2048])
                        at = phFac.tile([C, 2, 2048], BF16, tag="at4", name="at4")
                        for j in range(4):
                            row0 = 16 * st + 4 * j
                            for co in range(2):
                                ps = psum.tile([C, 512], F32, tag="ps", name="ps")
                                nc.tensor.matmul(
                                    ps, wcv2[:, 0, co * C:(co + 1) * C],
                                    yseg[:, j * 512:(j + 1) * 512],
                                    start=True, stop=False,
                                )
                                for ki, soff in ((1, 0), (2, 2080)):
                                    nc.tensor.matmul(
                                        ps, wcv2[:, ki, co * C:(co + 1) * C],
                                        _ap(ct, soff + 4 * j * HP + 1, [[HP, 4], [1, W]]),
                                        start=False, stop=False,
                                    )
                                nc.tensor.matmul(
                                    ps, wcv2[:, 3, co * C:(co + 1) * C],
                                    _ap(y2p, IOFF + row0 * HP, [[HP, 4], [1, W]]),
                                    start=False, stop=True,
                                )
                                nc.scalar.copy(at[:, co, j * 512:(j + 1) * 512], ps)
                        for co in range(2):
                            nc.sync.dma_start(
                                out=acc4_sp[co, :, st * 2048:(st + 1) * 2048],
                                in_=at[:, co, :],
                            )

                    # ---- reassembly. Softmax weights sum to 1 over taps, so
                    #   out_s = z00 + sum_{t!=00} w_t*(z_t - z00)
                    # with low-res tap diffs d_t precomputed once (tiny).
                    # Processed by (row-half, quarter q == subpos row r1) into a
                    # SPLIT y3pre map (A: hi-res rows -1..64, B: rows 63..128) so
                    # bneck3's first conv can start after the first half.
                    dts = {}
                    for t in (1, 2, 3):
                        ti, tj = t // 2, t % 2
                        dt = phF.tile([C, 1024], BF16, tag=f"d{t}", name=f"d{t}")
                        nc.vector.tensor_tensor(
                            out=dt,
                            in0=_ap(z3b if tj else z3a, ti * ZS, [[ZS, 32], [1, 32]]),
                            in1=_ap(z3a, 0, [[ZS, 32], [1, 32]]),
                            op=mybir.AluOpType.subtract,
                        )
                        dts[t] = dt
                    y3A = phFy3.tile([C, 66 * HP], F8, tag="y3A", name="y3A")
                    y3B = phFy3.tile([C, 66 * HP], F8, tag="y3B", name="y3B")
                    nc.vector.memset(_ap(y3A, 0, [[1, HP]]), 0.0)
                    nc.vector.memset(_ap(y3B, 65 * HP, [[1, HP]]), 0.0)
                    for mm in (y3A, y3B):
                        nc.vector.memset(_ap(mm, 0, [[HP, 66]]), 0.0)
                        nc.vector.memset(_ap(mm, HP - 1, [[HP, 66]]), 0.0)
                    wn_flat = wn_sp[:]
                    for half, q in ((1, 0), (0, 0), (0, 1), (0, 2), (0, 3),
                                    (1, 1), (1, 2), (1, 3)):
                        if True:
                            hoff = 16 * half
                            accq = phFacc.tile([C, 2048], BF16, tag="accq", name="accq")
                            dst = accq[:, :]
                            tmps = []
                            for t in (1, 2, 3):
                                ti, tj = t // 2, t % 2
                                wb = phFwb.tile([C, 2048], BF16, tag="wb", name="wb")
                                src2 = bass.AP(
                                    tensor=wn_flat.tensor,
                                    offset=wn_flat.offset + t * 16384 + q * 4096
                                    + half * 512,
                                    ap=[[0, C], [1024, 4], [1, 512]],
                                )
                                nc.gpsimd.dma_start(out=wb, in_=src2)
                                dread = _ap(dts[t], half * 512,
                                            [[0, 4], [32, 16], [1, 32]])
                                tmp = phFtmp.tile([C, 2048], BF16, tag=f"tmp{t % 2}",
                                                  name="tmp")
                                nc.vector.tensor_tensor(out=tmp, in0=wb, in1=dread,
                                                        op=mybir.AluOpType.mult)
                                tmps.append(tmp)
                                if t == 2:
                                    nc.vector.tensor_tensor(out=dst, in0=tmps[0],
                                                            in1=tmps[1],
                                                            op=mybir.AluOpType.add)
                            nc.vector.tensor_tensor(out=dst, in0=dst, in1=tmps[2],
                                                    op=mybir.AluOpType.add)
                            nc.vector.tensor_tensor(
                                out=dst, in0=dst,
                                in1=_ap(z3a, (hoff) * ZS, [[0, 4], [ZS, 16], [1, 32]]),
                                op=mybir.AluOpType.add,
                            )
                            # + y2 residual, rows R = 4h+q for h in this half
                            ymap = y3A if half == 0 else y3B
                            pos = [[1, 4], [4 * HP, 16], [4, 32]]
                            nc.vector.tensor_tensor(
                                out=_ap(ymap, (q + 1) * HP + 1, pos),
                                in0=accq[:, :],
                                in1=_ap(y2p, IOFF + (64 * half + q) * HP, pos),
                                op=mybir.AluOpType.add,
                            )
                            # boundary rows shared by both halves
                            if half == 0 and q == 3:  # R=63 -> B row 0
                                nc.vector.tensor_tensor(
                                    out=_ap(y3B, 1, [[1, 4], [4, 32]]),
                                    in0=_ap(accq, 15 * 32, [[512, 4], [1, 32]]),
                                    in1=_ap(y2p, IOFF + 63 * HP, [[1, 4], [4, 32]]),
                                    op=mybir.AluOpType.add,
                                )
                            if half == 1 and q == 0:  # R=64 -> A row 65
                                nc.vector.tensor_tensor(
                                    out=_ap(y3A, 65 * HP + 1, [[1, 4], [4, 32]]),
                                    in0=_ap(accq, 0, [[512, 4], [1, 32]]),
                                    in1=_ap(y2p, IOFF + 64 * HP, [[1, 4], [4, 32]]),
                                    op=mybir.AluOpType.add,
                                )

                # ===== Phase G: bneck3 + cv2 final, chained per 4-row tile =====
                with tc.tile_pool(name="phGy", bufs=4) as phGy, \
                     tc.tile_pool(name="phGc", bufs=2) as phGc, \
                     tc.tile_pool(name="phGo", bufs=2) as phGo:
                    t3p = maps.tile([C, HP * HP], F8, tag="m", name="t3p")
                    conv3x3_split(y3A, y3B, t3p, "m3a")

                    # m3b produces y3 in 4-row px-linear tiles; cv2 consumes each
                    # immediately (y3 is only ever read by cv2's 1x1 conv).
                    wt = wsb["m3b"]
                    s3, b3 = sb["m3b"][:, 0:1], sb["m3b"][:, 1:2]
                    ca = None
                    ot = None
                    for i in range(32):
                        st, jj = i // 4, i % 4
                        if jj == 0:
                            ca = phGc.tile([C, 2, 2048], BF16, tag="ca", name="ca")
                            nc.sync.dma_start(
                                out=ca, in_=acc4_sp[:, :, st * 2048:(st + 1) * 2048]
                                .rearrange("k p f -> p k f"))
                        if jj % 2 == 0:
                            ot = phGo.tile([C, 2, 1024], F32, tag="ot", name="ot")
                        ps = psum.tile([C, 512], F32, tag="ps", name="ps")
                        taps9_fp8(ps, wt, t3p, 4 * i * HP)
                        y3t = phGy.tile([C, 512], BF16, tag="y3t", name="y3t")
                        nc.scalar.activation(y3t, ps, AF.Silu, bias=b3, scale=s3)
                        for co in range(2):
                            ps2 = psum.tile([C, 512], F32, tag="ps", name="ps")
                            nc.tensor.matmul(
                                ps2, ident, ca[:, co, jj * 512:(jj + 1) * 512],
                                start=True, stop=False,
                            )
                            nc.tensor.matmul(
                                ps2, wcv2[:, 4, co * C:(co + 1) * C], y3t,
                                start=False, stop=True,
                            )
                            nc.scalar.activation(
                                ot[:, co, (jj % 2) * 512:(jj % 2 + 1) * 512], ps2,
                                AF.Silu,
                                bias=sb["cv2"][:, 2 + co:3 + co],
                                scale=sb["cv2"][:, co:co + 1],
                            )
                        if jj % 2 == 1:
                            base = st * 2048 + (jj // 2) * 1024
                            for co in range(2):
                                nc.sync.dma_start(
                                    out=out_d[co, :, base:base + 1024],
                                    in_=ot[:, co, :],
                                )
    return nc


def _bf(a):
    return np.ascontiguousarray(a.astype(ml_dtypes.bfloat16))


def _f8(a):
    assert np.abs(a).max() < 240.0, "fp8e4m3 overflow in weight prep"
    return np.ascontiguousarray(a.astype(ml_dtypes.float8_e4m3))


def prep_base_inputs(inp):
    """Host-side weight rearrangement -> the flat in_map (minus x)."""
    d = {}

    sball = np.zeros((C, 26), np.float32)
    sb_off = {"cv1": 0, "cv2": 4, "m1a": 8, "m1b": 10, "m2a": 12, "m2b": 14,
              "m3a": 16, "m3b": 18, "cvm2": 20, "cvm3": 22}

    def csb(pre, s, b, ntile):
        # scale/bias packed: cols [o:o+ntile]=scale, [o+ntile:o+2*ntile]=bias
        o = sb_off[pre]
        for i in range(ntile):
            sball[:, o + i] = s[i * C:(i + 1) * C]
            sball[:, o + ntile + i] = b[i * C:(i + 1) * C]

    # cv1: w [256, 256, 1, 1] -> [2 (ci tile), 128, 256 co]
    w = inp["cv1_w"][:, :, 0, 0]  # [co, ci]
    d["w_cv1"] = _bf(w.T.reshape(2, C, 2 * C))
    csb("cv1", inp["cv1_s"], inp["cv1_b"], 2)
    for name in ("m1a", "m1b", "m2a", "m2b", "m3a", "m3b"):
        w = inp[f"{name}_w"]  # [co, ci, 3, 3]
        d[f"w_{name}"] = _f8(np.transpose(w, (2, 3, 1, 0)).reshape(9, C, C) * WS)
        csb(name, inp[f"{name}_s"] / WS, inp[f"{name}_b"], 1)
    w = inp["cvm2_w"][:, :, 0, 0].reshape(C, C, 4)  # [co, c, ab]
    d["w_cvm2"] = _bf(np.transpose(w, (2, 1, 0)))  # [ab, ci, co]
    csb("cvm2", inp["cvm2_s"], inp["cvm2_b"], 1)
    w = inp["cvm3_w"][:, :, 0, 0].reshape(C, C, 16)
    d["w_cvm3"] = _bf(np.transpose(w, (2, 1, 0)))
    csb("cvm3", inp["cvm3_s"], inp["cvm3_b"], 1)
    w = inp["cv2_w"][:, :, 0, 0]  # [256, 640]
    d["w_cv2"] = _bf(w.T.reshape(5, C, 2 * C))
    csb("cv2", inp["cv2_s"], inp["cv2_b"], 2)
    d["w_dn"] = _bf(inp["u3_down_w"][:, :, 0, 0].T)  # [128 ci, 32]
    sball[0:32, 24] = inp["u3_down_b"].astype(np.float32)
    w = inp["u3_enc_w"]  # [64, 32, 2, 2]
    d["w_en"] = _bf(np.transpose(w, (2, 3, 1, 0)).reshape(4, 32, 64))
    sball[0:64, 25] = inp["u3_enc_b"].astype(np.float32)
    d["sball"] = sball
    i_idx = np.arange(64)
    d["s64"] = (i_idx[:, None] % 16 == i_idx[None, :] % 16).astype(np.float32)
    d["ident"] = _bf(np.eye(C, dtype=np.float32))
    return d


_NC_CACHE = {}
_TRACE = False  # test.py can flip this to capture an NTFF profile
_LAST_RESULT = None


def get_nc():
    if "nc" not in _NC_CACHE:
        nc = build_nc()
        nc.finalize()  # Bacc: run wait-splitting/reg-alloc passes before lowering
        _NC_CACHE["nc"] = nc
    return _NC_CACHE["nc"]


def make_in_maps(inputs):
    base = prep_base_inputs(inputs)
    x = inputs["x"]  # [8, 256, 128, 128] f32
    xb = _bf(x.reshape(N_CORES, 2, C, NPIX))
    return [dict(base, x=np.ascontiguousarray(xb[i])) for i in range(N_CORES)]


def kernel(**inputs):
    global _LAST_RESULT
    from concourse.bass_utils import run_bass_kernel_spmd

    nc = get_nc()
    in_maps = make_in_maps(inputs)
    res = run_bass_kernel_spmd(
        nc, in_maps, core_ids=list(range(N_CORES)), trace=_TRACE
    )
    _LAST_RESULT = res
    outs = [res.results[i]["out"].reshape(2 * C, H, W) for i in range(N_CORES)]
    return np.stack(outs).astype(np.float32)



# revision 50
# speedup vs baseline: 1.1563x; 1.1563x over previous
"""Trainium2 Bass kernel for the dense CNN (CSP block with CARAFE upsamplers).

Strategy: pure data parallelism — 8 samples over 8 NeuronCores, one sample per
core, full forward pass per core:

  cv1 (1x1, 256->256) -> split y / y0
  bneck1 = two 3x3 CBS convs (128->128)        [y1]
  cvm2 (1x1 over pixel_unshuffle(y1,2))        [z2, 64x64]
  carafe(k=1,up=2) == nearest-neighbor 2x upsample (softmax over a single
      channel is identically 1 -> its down/enc convs are dead code) + y1
  bneck2                                        [y2]
  cvm3 (1x1 over pixel_unshuffle(y2,4))        [z3, 32x32]
  carafe(k=2,up=4): per-pixel 2x2-tap softmax weights, 4x up + y2  [y3pre]
  bneck3                                        [y3]
  cv2 (1x1, 640->256) over concat(y, y0, y1, y2, y3)

Implementation notes:
- Feature maps live in SBUF as [128 ch, 130*130] bf16 with a zero halo; a 3x3
  conv is 9 shifted-tap matmuls accumulated in fp32 PSUM (bf16 streams at the
  PE's full 1 elem/cell/cycle rate); each conv epilogue is one ScalarE op:
  silu(psum*scale + bias) with per-partition f32 scale/bias.
- pixel_unshuffle feeds the 1x1 convs as strided-AP k-slices (host-side
  weight re-layout), so no data movement is spent on it.
- CARAFE k=2/up=4: softmax weights computed on-chip ([64,1024], exp on
  ScalarE, tap-sum via a mod-16 indicator matmul), bounced to DRAM, and
  partition-broadcast back per (row-half, subpos-row) chunk via SWDGE DMA.
  Since the weights sum to 1, the 4-tap blend is z00 + sum w_t*(z_t - z00)
  (3 multiplies instead of 4) on the DVE at the bf16 2x rate, with an even
  (34) row stride + a column-shifted copy of z3 to keep reads 4B-aligned.
- cv2 (640->256, 1x1) is split: k-tiles {y,y0,y1,y2} are computed as a bf16
  partial (acc4, bounced through DRAM) on the PE during the CARAFE DVE window;
  the y3 k-tile + an identity-matmul re-injection of acc4 are chained per
  4-row tile directly behind bneck3's second conv, so PE never waits on a
  whole-map dependency at the end.
- y3pre is built as two half-maps (rows -1..64 / 63..128) so bneck3's first
  conv starts while the second half of the reassembly is still on the DVE.
- y/y0/y1 spill to DRAM (bf16) and stream back for the cv2 partial; all
  weights are pre-transposed/pre-sliced and pre-cast to bf16 on the host.

Next identified (unshipped) optimization: split y2 into two halo-free
half-maps (rows 0..63 / 64..127). No 3x3 conv reads y2 -- only cvm3,
the cv2 partial, and the CARAFE residual, all of which read within one
half -- so the split needs no boundary duplication, and cvm3's first
tile (+ the whole kernel-prediction chain gating the CARAFE DVE work)
could start ~25us earlier, during bneck2's second half. Beyond that,
the remaining ~55us over the matmul roofline is: the fixed ~6us drain
barrier, ~6us startup DMA latency, small-N matmul inefficiency in
cvm2/cvm3/kt, and boundary stalls whose measured fixes all regressed.
fp8+DoubleRow on the six 3x3 convs is the only large remaining lever
(~2x PE peak) and requires an accuracy-gate revalidation.
"""
import sys

sys.path.insert(0, "/opt/trn_rl_repo")

import numpy as np
import ml_dtypes

import concourse.bass as bass
import concourse.bacc as bacc
import concourse.mybir as mybir
import concourse.tile as tile

F32 = mybir.dt.float32
BF16 = mybir.dt.bfloat16
F8 = mybir.dt.float8e4
AF = mybir.ActivationFunctionType
DR = mybir.MatmulPerfMode.DoubleRow
WS = 64.0  # fp8 3x3-conv weight pre-scale (folded out via the BN scale)

N_CORES = 8
C = 128
H = W = 128
HP = H + 2  # padded row length
NPIX = H * W
IOFF = HP + 1  # offset of interior (1,1) in padded layout
ZP = 33  # z3 logical padded side (rows/cols -1..31)
ZS = 34  # z3 row stride (even, for DVE 2x alignment)
TAPS9 = [(dy, dx) for dy in (-1, 0, 1) for dx in (-1, 0, 1)]


def _ap(t, off, dims):
    """Free-dim AP into tile t (keeps full partition dim)."""
    return bass.AP(tensor=t.tensor, offset=t.offset + off, ap=[list(t.ap[0])] + dims)


def _zero_border(nc, mp, side=HP):
    nc.vector.memset(_ap(mp, 0, [[1, side]]), 0.0)
    nc.vector.memset(_ap(mp, (side - 1) * side, [[1, side]]), 0.0)
    nc.vector.memset(_ap(mp, side, [[side, side - 2]]), 0.0)
    nc.vector.memset(_ap(mp, side + side - 1, [[side, side - 2]]), 0.0)


def build_nc():
    nc = bacc.Bacc(None)

    # ---- I/O ----
    x_d = nc.dram_tensor("x", [2, C, NPIX], BF16, kind="ExternalInput")
    w_m = {}
    for name in ("m1a", "m1b", "m2a", "m2b", "m3a", "m3b"):
        w_m[name] = nc.dram_tensor(f"w_{name}", [10, C, C], F8, kind="ExternalInput")
    w_cv1 = nc.dram_tensor("w_cv1", [2, C, 2 * C], BF16, kind="ExternalInput")
    w_cvm2 = nc.dram_tensor("w_cvm2", [4, C, C], BF16, kind="ExternalInput")
    w_cvm3 = nc.dram_tensor("w_cvm3", [16, C, C], BF16, kind="ExternalInput")
    w_cv2 = nc.dram_tensor("w_cv2", [5, C, 2 * C], BF16, kind="ExternalInput")
    w_dn = nc.dram_tensor("w_dn", [C, 32], BF16, kind="ExternalInput")
    w_en = nc.dram_tensor("w_en", [4, 32, 64], BF16, kind="ExternalInput")
    s64_d = nc.dram_tensor("s64", [64, 64], F32, kind="ExternalInput")
    id_d = nc.dram_tensor("ident", [C, C], BF16, kind="ExternalInput")
    sball_d = nc.dram_tensor("sball", [C, 26], F32, kind="ExternalInput")
    out_d = nc.dram_tensor("out", [2, C, NPIX], BF16, kind="ExternalOutput")

    # ---- DRAM scratch ----
    y_sp = nc.dram_tensor("y_sp", [C, NPIX], BF16)  # px-linear
    pad_sp = {k: nc.dram_tensor(f"{k}_sp", [C, HP * HP], BF16) for k in ("y0", "y1")}
    wn_sp = nc.dram_tensor("wn_sp", [64 * 1024], BF16)
    acc4_sp = nc.dram_tensor("acc4_sp", [2, C, NPIX], BF16)  # cv2 partial (y,y0,y1,y2)

    with tile.TileContext(nc) as tc:
        import contextlib

        est = contextlib.ExitStack()
        with est:
            consts = est.enter_context(tc.tile_pool(name="consts", bufs=1))
            maps = est.enter_context(tc.tile_pool(name="maps", bufs=2))
            psum = est.enter_context(tc.tile_pool(name="psum", bufs=2, space="PSUM"))
            # cv2-partial stream tiles (y0,y1 padded row-blocks); opened at top
            # level so stages 0-3 can prefetch during bneck2's PE-bound window
            phFpre = est.enter_context(tc.tile_pool(name="phFpre", bufs=3))
            phFy = est.enter_context(tc.tile_pool(name="phFy", bufs=2))
            cts = {}

            def load_ct(st):
                ct = phFpre.tile([C, 2 * 2080], BF16, tag="ct4", name="ct4")
                poff = (st * 16 + 1) * HP
                nc.sync.dma_start(out=ct[:, 0:2080],
                                  in_=pad_sp["y0"][:, poff:poff + 2080])
                nc.sync.dma_start(out=ct[:, 2080:2 * 2080],
                                  in_=pad_sp["y1"][:, poff:poff + 2080])
                cts[st] = ct

            # ---- constants (small, plus cv2 weights + identity: persistent) ----
            sball = consts.tile([C, 26], F32, tag="sball", name="sball")
            nc.sync.dma_start(out=sball, in_=sball_d[:, :])
            sb = {}
            off = 0
            for name, w_ in (("cv1", 4), ("cv2", 4), ("m1a", 2), ("m1b", 2),
                             ("m2a", 2), ("m2b", 2), ("m3a", 2), ("m3b", 2),
                             ("cvm2", 2), ("cvm3", 2)):
                sb[name] = sball[:, off:off + w_]
                off += w_
            sb["dn_b"] = sball[0:32, 24:25]
            sb["en_b"] = sball[0:64, 25:26]

            s64 = consts.tile([64, 64], F32, tag="s64")
            nc.sync.dma_start(out=s64, in_=s64_d[:, :])
            wdn = consts.tile([C, 32], BF16, tag="wdn")
            nc.sync.dma_start(out=wdn, in_=w_dn[:, :])
            wen = consts.tile([32, 4, 64], BF16, tag="wen")
            nc.sync.dma_start(out=wen, in_=w_en[:, :, :].rearrange("t p n -> p t n"))
            ident = consts.tile([C, C], BF16, tag="ident")
            nc.sync.dma_start(out=ident, in_=id_d[:, :])
            wcv2 = consts.tile([C, 5, 2 * C], BF16, tag="wcv2")
            nc.sync.dma_start(out=wcv2, in_=w_cv2[:, :, :].rearrange("t p n -> p t n"))
            wsb = {}

            def load_w(pool, name, src, shape, dt=BF16):
                t = pool.tile(shape, dt, tag=f"w_{name}", name=f"w_{name}")
                nc.sync.dma_start(out=t, in_=src[:, :, :].rearrange("t p n -> p t n"))
                wsb[name] = t

            # 10 tap slots: [t0..t3, t4/2, t5, t6, t7, t4/2, t8] -- the
            # center tap is split into two exactly-halved fp8 copies so all
            # five pairs run in DoubleRow mode (no full-rate single tap).
            TAPS10 = [(-1, -1), (-1, 0), (-1, 1), (0, -1), (0, 0),
                      (0, 1), (1, -1), (1, 0), (0, 0), (1, 1)]

            def taps9_fp8(ps, wt, src, off0):
                """9-tap accumulation as 5 fp8 DoubleRow pair-matmuls;
                off0 = offset of the (dy=-1,dx=-1) tap window."""
                for p in range(5):
                    dy0, dx0 = TAPS10[2 * p]
                    dy1, dx1 = TAPS10[2 * p + 1]
                    delta = (dy1 - dy0) * HP + (dx1 - dx0)
                    rhs = bass.AP(
                        tensor=src.tensor,
                        offset=src.offset + off0 + (dy0 + 1) * HP + (dx0 + 1),
                        ap=[list(src.ap[0]), [delta, 2], [HP, 4], [1, W]],
                    )
                    nc.tensor.matmul(ps, wt[:, 2 * p:2 * p + 2, :], rhs,
                                     start=(p == 0), stop=(p == 4), perf_mode=DR)

            def conv3x3_split(srcA, srcB, dst, wname, i0=0, i1=16,
                                  border=True):
                """CBS 3x3 whose input lives in two row-half maps (66 rows each:
                A = hi-res rows -1..64, B = rows 63..128)."""
                wt = wsb[wname]
                s_ap, b_ap = sb[wname][:, 0:1], sb[wname][:, 1:2]
                if border:
                    _zero_border(nc, dst)
                for i in range(i0, i1):
                    srcm = srcA if i < 8 else srcB
                    ii = i if i < 8 else i - 8
                    ps = psum.tile([C, 1024], F32, tag="ps", name="ps")
                    for h in range(2):
                        taps9_fp8(ps[:, h * 512:(h + 1) * 512], wt, srcm,
                                  (8 * ii + 4 * h) * HP)
                    nc.scalar.activation(
                        _ap(dst, IOFF + 8 * i * HP, [[HP, 8], [1, W]]),
                        ps[:, :].rearrange("p (r w) -> p r w", r=8),
                        AF.Silu, bias=b_ap, scale=s_ap,
                    )

            def conv3x3(src, dst, wname, i0=0, i1=16, border=True):
                """CBS 3x3: src padded map -> dst padded map (interior)."""
                wt = wsb[wname]
                s_ap, b_ap = sb[wname][:, 0:1], sb[wname][:, 1:2]
                if border:
                    _zero_border(nc, dst)
                for i in range(i0, i1):
                    ps = psum.tile([C, 1024], F32, tag="ps", name="ps")
                    for h in range(2):
                        taps9_fp8(ps[:, h * 512:(h + 1) * 512], wt, src,
                                  (8 * i + 4 * h) * HP)
                    nc.scalar.activation(
                        _ap(dst, IOFF + 8 * i * HP, [[HP, 8], [1, W]]),
                        ps[:, :].rearrange("p (r w) -> p r w", r=8),
                        AF.Silu, bias=b_ap, scale=s_ap,
                    )

            with tc.tile_pool(name="wearly", bufs=1) as wearly:
                load_w(consts, "cv1", w_cv1, [C, 2, 2 * C])

                # ================= Phase A: cv1 =================
                y0p = maps.tile([C, HP * HP], BF16, tag="m")
                _zero_border(nc, y0p)
                # fp8 shadow of y0 for bneck1 (cv2/spill keep the bf16 map);
                # DVE copies chase cv1's stages in the otherwise-idle window
                y0p8 = wearly.tile([C, HP * HP], F8, tag="y08")
                _zero_border(nc, y0p8)
                with tc.tile_pool(name="phA", bufs=3) as phA, \
                     tc.tile_pool(name="phAy", bufs=3) as phAy:
                    for st in range(8):  # stages of 2048 px (16 rows)
                        xt = phA.tile([C, 2, 2048], BF16, tag="xt", name="xt")
                        for kt in range(2):
                            nc.sync.dma_start(
                                out=xt[:, kt, :], in_=x_d[kt, :, st * 2048:(st + 1) * 2048]
                            )
                        yt = phAy.tile([C, 2048], BF16, tag="yt", name="yt")
                        for jp in range(2):
                            row0 = 16 * st + 8 * jp
                            psy = psum.tile([C, 1024], F32, tag="ps", name="ps")
                            psy0 = psum.tile([C, 1024], F32, tag="ps", name="ps")
                            for h in range(2):
                                xs = xt[:, :, (2 * jp + h) * 512:(2 * jp + h + 1) * 512]
                                for kt in range(2):
                                    nc.tensor.matmul(
                                        psy[:, h * 512:(h + 1) * 512],
                                        wsb["cv1"][:, kt, 0:C], xs[:, kt, :],
                                        start=(kt == 0), stop=(kt == 1),
                                    )
                                for kt in range(2):
                                    nc.tensor.matmul(
                                        psy0[:, h * 512:(h + 1) * 512],
                                        wsb["cv1"][:, kt, C:2 * C], xs[:, kt, :],
                                        start=(kt == 0), stop=(kt == 1),
                                    )
                            nc.scalar.activation(
                                yt[:, jp * 1024:(jp + 1) * 1024], psy, AF.Silu,
                                bias=sb["cv1"][:, 2:3], scale=sb["cv1"][:, 0:1],
                            )
                            nc.scalar.activation(
                                _ap(y0p, IOFF + row0 * HP, [[HP, 8], [1, W]]),
                                psy0[:, :].rearrange("p (r w) -> p r w", r=8),
                                AF.Silu, bias=sb["cv1"][:, 3:4], scale=sb["cv1"][:, 1:2],
                            )
                        nc.sync.dma_start(
                            out=y_sp[:, st * 2048:(st + 1) * 2048], in_=yt
                        )
                        nc.vector.tensor_copy(
                            _ap(y0p8, IOFF + 16 * st * HP, [[HP, 16], [1, W]]),
                            _ap(y0p, IOFF + 16 * st * HP, [[HP, 16], [1, W]]),
                        )
                nc.sync.dma_start(out=pad_sp["y0"][:, :], in_=y0p)

                # remaining early weights (loads overlap with phase A/B compute)
                load_w(consts, "m1a", w_m["m1a"], [C, 10, C], dt=F8)
                load_w(consts, "m1b", w_m["m1b"], [C, 10, C], dt=F8)
                load_w(consts, "cvm2", w_cvm2, [C, 4, C])
                load_w(consts, "m2a", w_m["m2a"], [C, 10, C], dt=F8)
                load_w(consts, "m2b", w_m["m2b"], [C, 10, C], dt=F8)

                # ================= Phase B: bneck1 =================
                with tc.tile_pool(name="phB", bufs=1) as phB:
                    t1p = phB.tile([C, HP * HP], F8, tag="aux", name="t1p")
                    conv3x3(y0p8, t1p, "m1a")
                    y1p = maps.tile([C, HP * HP], BF16, tag="m")
                    conv3x3(t1p, y1p, "m1b")
            nc.sync.dma_start(out=pad_sp["y1"][:, :], in_=y1p)
            load_w(consts, "cvm3", w_cvm3, [C, 16, C])
            load_w(consts, "m3a", w_m["m3a"], [C, 10, C], dt=F8)
            load_w(consts, "m3b", w_m["m3b"], [C, 10, C], dt=F8)

            # ================= Phase C/D: cvm2 + carafe2 =================
            for st in range(3):
                load_ct(st)
            y2pp = maps.tile([C, HP * HP], F8, tag="m")
            _zero_border(nc, y2pp)
            for half in range(2):
                z2 = phFy.tile([C, 2048], BF16, tag="yseg", name="z2")
                for ip in range(2):  # 16 z2-rows of 64 px per psum tile
                    ps = psum.tile([C, 1024], F32, tag="ps", name="ps")
                    for h in range(2):
                        i = 2 * ip + h
                        for abi in range(4):
                            a, b = abi // 2, abi % 2
                            rhs = _ap(y1p, ((64 * half + 16 * i) + a + 1) * HP + (b + 1),
                                      [[2 * HP, 8], [2, 64]])
                            nc.tensor.matmul(ps[:, h * 512:(h + 1) * 512],
                                             wsb["cvm2"][:, abi, :], rhs,
                                             start=(abi == 0), stop=(abi == 3))
                    nc.scalar.activation(
                        z2[:, ip * 1024:(ip + 1) * 1024], ps, AF.Silu,
                        bias=sb["cvm2"][:, 1:2], scale=sb["cvm2"][:, 0:1],
                    )
                    # carafe2 == NN 2x upsample + y1, per 16-z2-row group so
                    # m2a's first tiles unblock right after the first group
                    for abi in range(4):
                        a, b = abi // 2, abi % 2
                        pos = [[2 * HP, 16], [2, 64]]
                        roff = (64 * half + 32 * ip + a) * HP + b
                        nc.vector.tensor_tensor(
                            out=_ap(y2pp, IOFF + roff, pos),
                            in0=z2[:, ip * 1024:(ip + 1) * 1024]
                            .rearrange("p (h w) -> p h w", h=16),
                            in1=_ap(y1p, IOFF + roff, pos),
                            op=mybir.AluOpType.add,
                        )

            # ==== Phases E+F: bneck2 + cvm3/carafe4 + cv2 partial ====
            # The CARAFE kernel-prediction chain is interleaved between m2b's
            # two row-halves: region-granular tile deps let cvm3-h0 + the
            # down/enc/softmax chain run on PE/ScalarE/DVE/DMA during m2b's
            # PE-bound second half, so the reassembly DVE block starts almost
            # immediately after m2b (previously ~13us of serial kt latency).
            with tc.tile_pool(name="phFy3", bufs=1) as phFy3:
                with tc.tile_pool(name="phE", bufs=1) as phE, \
                     tc.tile_pool(name="phF", bufs=1) as phF, \
                     tc.tile_pool(name="phFwb", bufs=2) as phFwb, \
                     tc.tile_pool(name="phFtmp", bufs=1) as phFtmp, \
                     tc.tile_pool(name="phFacc", bufs=2) as phFacc, \
                     tc.tile_pool(name="phFac", bufs=3) as phFac, \
                     tc.tile_pool(name="psF", bufs=2, space="PSUM") as psF:
                    t2p = phE.tile([C, HP * HP], F8, tag="aux", name="t2p")
                    conv3x3(y2pp, t2p, "m2a")
                    y2p = maps.tile([C, HP * HP], BF16, tag="m")

                    # ---- F-prep tiles; memsets run early on the idle DVE ----
                    z3a = phF.tile([C, ZS * ZP], BF16, tag="z3a", name="z3a")
                    nc.vector.memset(z3a, 0.0)
                    z3b = phF.tile([C, ZS * ZP], BF16, tag="z3b", name="z3b")
                    nc.vector.memset(z3b, 0.0)
                    kt1p = phF.tile([32, ZP * ZP], BF16, tag="kt1p", name="kt1p")
                    _zero_border(nc, kt1p, side=ZP)
                    e_sb = phF.tile([64, 1024], F32, tag="e", name="e_sb")
                    rden = phF.tile([64, 1024], F32, tag="rden", name="rden")
                    wnt = phF.tile([64, 1024], BF16, tag="wnt", name="wnt")
                    y3A = phFy3.tile([C, 66 * HP], F8, tag="y3A", name="y3A")
                    y3B = phFy3.tile([C, 66 * HP], F8, tag="y3B", name="y3B")
                    nc.vector.memset(_ap(y3A, 0, [[1, HP]]), 0.0)
                    nc.vector.memset(_ap(y3B, 65 * HP, [[1, HP]]), 0.0)
                    for mm in (y3A, y3B):
                        nc.vector.memset(_ap(mm, 0, [[HP, 66]]), 0.0)
                        nc.vector.memset(_ap(mm, HP - 1, [[HP, 66]]), 0.0)
                    wn_flat = wn_sp[:]

                    def kt_chain():
                        """cvm3 + CARAFE kernel prediction, baseline form."""
                        for i in range(2):  # 16 z3-rows of 32 px per tile
                            psz = psum.tile([C, 512], F32, tag="ps", name="psz")
                            for abi in range(16):
                                a, b = abi // 4, abi % 4
                                rhs = _ap(y2p, (64 * i + a + 1) * HP + (b + 1),
                                          [[4 * HP, 16], [4, 32]])
                                nc.tensor.matmul(psz, wsb["cvm3"][:, abi, :], rhs,
                                                 start=(abi == 0), stop=(abi == 15))
                            nc.scalar.activation(
                                _ap(z3a, (16 * i + 1) * ZS + 1, [[ZS, 16], [1, 32]]),
                                psz[:, :].rearrange("p (r w) -> p r w", r=16),
                                AF.Silu, bias=sb["cvm3"][:, 1:2],
                                scale=sb["cvm3"][:, 0:1],
                            )
                        for i in range(2):
                            ps1 = psF.tile([32, 512], F32, tag="psk", name="psk")
                            nc.tensor.matmul(
                                ps1, wdn,
                                _ap(z3a, (16 * i + 1) * ZS + 1, [[ZS, 16], [1, 32]]),
                                start=True, stop=True,
                            )
                            nc.scalar.activation(
                                _ap(kt1p, (16 * i + 1) * ZP + 1, [[ZP, 16], [1, 32]]),
                                ps1[:, :].rearrange("p (r w) -> p r w", r=16),
                                AF.Identity, bias=sb["dn_b"][:, 0:1],
                            )
                        for i in range(2):
                            ps2 = psF.tile([64, 512], F32, tag="psk", name="psk")
                            for t in range(4):
                                di, dj = t // 2, t % 2
                                rhs = bass.AP(
                                    tensor=kt1p.tensor,
                                    offset=kt1p.offset + (16 * i + di) * ZP + dj,
                                    ap=[list(kt1p.ap[0]), [ZP, 16], [1, 32]],
                                )
                                nc.tensor.matmul(ps2, wen[:, t, :], rhs,
                                                 start=(t == 0), stop=(t == 3))
                            nc.scalar.activation(
                                e_sb[:, i * 512:(i + 1) * 512],
                                ps2[:, :].rearrange("p (r w) -> p r w", r=16),
                                AF.Exp, bias=sb["en_b"][:, 0:1],
                            )
                        sm = psF.tile([64, 1024], F32, tag="psk", name="psk")
                        for i in range(2):
                            nc.tensor.matmul(sm[:, i * 512:(i + 1) * 512], s64,
                                             e_sb[:, i * 512:(i + 1) * 512],
                                             start=True, stop=True)
                        nc.vector.reciprocal(rden, sm)
                        nc.vector.tensor_tensor(out=wnt, in0=e_sb, in1=rden,
                                                op=mybir.AluOpType.mult)
                        nc.sync.dma_start(
                            out=wn_sp[:].rearrange("(p f) -> p f", p=64), in_=wnt)

                    # ---- m2b with the kt chain slotted between the halves ----
                    conv3x3(t2p, y2p, "m2b")
                    load_ct(3)
                    kt_chain()

                    # column-shifted copy (even tap starts) + low-res tap diffs
                    nc.vector.tensor_copy(
                        _ap(z3b, 0, [[ZS, ZP], [1, ZP - 1]]),
                        _ap(z3a, 1, [[ZS, ZP], [1, ZP - 1]]),
                    )
                    dts = {}
                    for t in (1, 2, 3):
                        ti, tj = t // 2, t % 2
                        dt = phF.tile([C, 1024], BF16, tag=f"d{t}", name=f"d{t}")
                        nc.vector.tensor_tensor(
                            out=dt,
                            in0=_ap(z3b if tj else z3a, ti * ZS, [[ZS, 32], [1, 32]]),
                            in1=_ap(z3a, 0, [[ZS, 32], [1, 32]]),
                            op=mybir.AluOpType.subtract,
                        )
                        dts[t] = dt

                    # ---- cv2 partial: acc4 = w_y.y + w_y0.y0 + w_y1.y1 + w_y2.y2
                    # (PE bulk that runs concurrently with the DVE reassembly)
                    kept = {}
                    ysegs = {}

                    def load_yseg(st):
                        yseg = phFy.tile([C, 2048], BF16, tag="yseg", name="yseg")
                        nc.sync.dma_start(out=yseg,
                                          in_=y_sp[:, st * 2048:(st + 1) * 2048])
                        ysegs[st] = yseg

                    load_yseg(0)
                    for st in range(8):
                        if st >= 4:
                            load_ct(st)
                        if st < 7:
                            load_yseg(st + 1)
                        ct = cts.pop(st)
                        yseg = ysegs.pop(st)
                        if st >= 6:
                            # last 2 stages stay in SBUF (phFpre ring is free by
                            # now); phase G reads them directly, skipping DRAM
                            atk = phFpre.tile([C, 2, 2048], BF16, tag="ct4",
                                              name="atk")
                            kept[st] = atk
                        for jp in range(2):
                            for co in range(2):
                                ps = psum.tile([C, 1024], F32, tag="ps", name="ps")
                                for h in range(2):
                                    j = 2 * jp + h
                                    pss = ps[:, h * 512:(h + 1) * 512]
                                    nc.tensor.matmul(
                                        pss, wcv2[:, 0, co * C:(co + 1) * C],
                                        yseg[:, j * 512:(j + 1) * 512],
                                        start=True, stop=False,
                                    )
                                    for ki, soff in ((1, 0), (2, 2080)):
                                        nc.tensor.matmul(
                                            pss, wcv2[:, ki, co * C:(co + 1) * C],
                                            _ap(ct, soff + 4 * j * HP + 1,
                                                [[HP, 4], [1, W]]),
                                            start=False, stop=False,
                                        )
                                    nc.tensor.matmul(
                                        pss, wcv2[:, 3, co * C:(co + 1) * C],
                                        _ap(y2p, IOFF + (16 * st + 4 * j) * HP,
                                            [[HP, 4], [1, W]]),
                                        start=False, stop=True,
                                    )
                                if st >= 6:
                                    nc.scalar.copy(
                                        kept[st][:, co, jp * 1024:(jp + 1) * 1024],
                                        ps)
                                else:
                                    att = phFac.tile([C, 1024], BF16, tag="at4",
                                                     name="att")
                                    nc.scalar.copy(att, ps)
                                    nc.sync.dma_start(
                                        out=acc4_sp[co, :,
                                                    st * 2048 + jp * 1024:
                                                    st * 2048 + (jp + 1) * 1024],
                                        in_=att)

                    # ---- reassembly: out_s = z00 + sum_{t!=00} w_t*(z_t - z00),
                    # half-0 chunks first (their weights bounced during m2b);
                    # half-1's normalization interleaves behind two chunks.
                    for half, q in ((0, 0), (0, 1), (0, 2), (0, 3),
                                    (1, 0), (1, 1), (1, 2), (1, 3)):
                        hoff = 16 * half
                        accq = phFacc.tile([C, 2048], BF16, tag="accq", name="accq")
                        dst = accq[:, :]
                        for t in (1, 2, 3):
                            wb = phFwb.tile([C, 2048], BF16, tag="wb", name="wb")
                            src2 = bass.AP(
                                tensor=wn_flat.tensor,
                                offset=wn_flat.offset + t * 16384 + q * 4096
                                + half * 512,
                                ap=[[0, C], [1024, 4], [1, 512]],
                            )
                            nc.gpsimd.dma_start(out=wb, in_=src2)
                            dread = _ap(dts[t], half * 512,
                                        [[0, 4], [32, 16], [1, 32]])
                            if t == 1:
                                nc.vector.tensor_tensor(out=dst, in0=wb, in1=dread,
                                                        op=mybir.AluOpType.mult)
                            else:
                                tmp = phFtmp.tile([C, 2048], BF16, tag="tmp",
                                                  name="tmp")
                                nc.vector.tensor_tensor(out=tmp, in0=wb, in1=dread,
                                                        op=mybir.AluOpType.mult)
                                nc.vector.tensor_tensor(out=dst, in0=dst, in1=tmp,
                                                        op=mybir.AluOpType.add)
                        nc.vector.tensor_tensor(
                            out=dst, in0=dst,
                            in1=_ap(z3a, (hoff) * ZS, [[0, 4], [ZS, 16], [1, 32]]),
                            op=mybir.AluOpType.add,
                        )
                        # + y2 residual, rows R = 4h+q for h in this half
                        ymap = y3A if half == 0 else y3B
                        pos = [[1, 4], [4 * HP, 16], [4, 32]]
                        nc.vector.tensor_tensor(
                            out=_ap(ymap, (q + 1) * HP + 1, pos),
                            in0=accq[:, :],
                            in1=_ap(y2p, IOFF + (64 * half + q) * HP, pos),
                            op=mybir.AluOpType.add,
                        )
                        # boundary rows shared by both halves
                        if half == 0 and q == 3:  # R=63 -> B row 0
                            nc.vector.tensor_tensor(
                                out=_ap(y3B, 1, [[1, 4], [4, 32]]),
                                in0=_ap(accq, 15 * 32, [[512, 4], [1, 32]]),
                                in1=_ap(y2p, IOFF + 63 * HP, [[1, 4], [4, 32]]),
                                op=mybir.AluOpType.add,
                            )
                        if half == 1 and q == 0:  # R=64 -> A row 65
                            nc.vector.tensor_tensor(
                                out=_ap(y3A, 65 * HP + 1, [[1, 4], [4, 32]]),
                                in0=_ap(accq, 0, [[512, 4], [1, 32]]),
                                in1=_ap(y2p, IOFF + 64 * HP, [[1, 4], [4, 32]]),
                                op=mybir.AluOpType.add,
                            )

                # ===== Phase G: bneck3 + cv2 final, chained per 4-row tile =====
                with tc.tile_pool(name="phGy", bufs=3) as phGy, \
                     tc.tile_pool(name="phGc", bufs=2) as phGc, \
                     tc.tile_pool(name="phGa", bufs=3) as phGa, \
                     tc.tile_pool(name="phGo", bufs=2) as phGo, \
                     tc.tile_pool(name="psG", bufs=2, space="PSUM") as psG:
                    t3p = maps.tile([C, HP * HP], F8, tag="m", name="t3p")

                    # m3b produces y3 in 8-row px-linear tiles; cv2 consumes each
                    # immediately (y3 is only ever read by cv2's 1x1 conv). The
                    # acc4 partial is re-injected on the DVE (idle here), not via
                    # an identity matmul, so the PE only runs m3b + the y3 k-tile.
                    # m3b tiles 0-6 (which only read t3p rows <= 56) are emitted
                    # between m3a's halves so the scalar pipeline starts early.
                    wt = wsb["m3b"]
                    s3, b3 = sb["m3b"][:, 0:1], sb["m3b"][:, 1:2]
                    ca = None

                    def m3b_tile(i):
                        nonlocal ca
                        st, jj = i // 2, i % 2
                        if jj == 0:
                            if st in kept:
                                ca = kept[st]
                            else:
                                ca = phGc.tile([C, 2, 2048], BF16, tag="ca",
                                               name="ca")
                                nc.sync.dma_start(
                                    out=ca,
                                    in_=acc4_sp[:, :, st * 2048:(st + 1) * 2048]
                                    .rearrange("k p f -> p k f"))
                        ot = phGo.tile([C, 2, 1024], BF16, tag="ot", name="ot")
                        ps = psum.tile([C, 1024], F32, tag="ps", name="ps")
                        for h in range(2):
                            taps9_fp8(ps[:, h * 512:(h + 1) * 512], wt, t3p,
                                      (8 * i + 4 * h) * HP)
                        y3t = phGy.tile([C, 1024], BF16, tag="y3t", name="y3t")
                        nc.scalar.activation(y3t, ps, AF.Silu, bias=b3, scale=s3)
                        for co in range(2):
                            ps2 = psG.tile([C, 1024], F32, tag="ps2", name="ps2")
                            for h in range(2):
                                nc.tensor.matmul(
                                    ps2[:, h * 512:(h + 1) * 512],
                                    wcv2[:, 4, co * C:(co + 1) * C],
                                    y3t[:, h * 512:(h + 1) * 512],
                                    start=True, stop=True,
                                )
                            sm_t = phGa.tile([C, 1024], F32, tag="sm", name="sm_t")
                            nc.vector.tensor_tensor(
                                out=sm_t, in0=ps2,
                                in1=ca[:, co, jj * 1024:(jj + 1) * 1024],
                                op=mybir.AluOpType.add,
                            )
                            nc.scalar.activation(
                                ot[:, co, :], sm_t, AF.Silu,
                                bias=sb["cv2"][:, 2 + co:3 + co],
                                scale=sb["cv2"][:, co:co + 1],
                            )
                        base = st * 2048 + jj * 1024
                        for co in range(2):
                            nc.sync.dma_start(
                                out=out_d[co, :, base:base + 1024],
                                in_=ot[:, co, :],
                            )

                    conv3x3_split(y3A, y3B, t3p, "m3a", i0=0, i1=8)
                    for i in range(7):
                        m3b_tile(i)
                    conv3x3_split(y3A, y3B, t3p, "m3a", i0=8, i1=16, border=False)
                    for i in range(7, 16):
                        m3b_tile(i)
    return nc


def _bf(a):
    return np.ascontiguousarray(a.astype(ml_dtypes.bfloat16))


def _f8(a):
    assert np.abs(a).max() < 240.0, "fp8e4m3 overflow in weight prep"
    return np.ascontiguousarray(a.astype(ml_dtypes.float8_e4m3))


def prep_base_inputs(inp):
    """Host-side weight rearrangement -> the flat in_map (minus x)."""
    d = {}

    sball = np.zeros((C, 26), np.float32)
    sb_off = {"cv1": 0, "cv2": 4, "m1a": 8, "m1b": 10, "m2a": 12, "m2b": 14,
              "m3a": 16, "m3b": 18, "cvm2": 20, "cvm3": 22}

    def csb(pre, s, b, ntile):
        # scale/bias packed: cols [o:o+ntile]=scale, [o+ntile:o+2*ntile]=bias
        o = sb_off[pre]
        for i in range(ntile):
            sball[:, o + i] = s[i * C:(i + 1) * C]
            sball[:, o + ntile + i] = b[i * C:(i + 1) * C]

    # cv1: w [256, 256, 1, 1] -> [2 (ci tile), 128, 256 co]
    w = inp["cv1_w"][:, :, 0, 0]  # [co, ci]
    d["w_cv1"] = _bf(w.T.reshape(2, C, 2 * C))
    csb("cv1", inp["cv1_s"], inp["cv1_b"], 2)
    for name in ("m1a", "m1b", "m2a", "m2b", "m3a", "m3b"):
        w = inp[f"{name}_w"]  # [co, ci, 3, 3]
        w9 = np.transpose(w, (2, 3, 1, 0)).reshape(9, C, C) * WS
        # 10 slots: center tap exactly halved into slots 4 and 8 so all five
        # DoubleRow pairs are well-formed (fp8 halving is exact)
        w10 = np.stack([w9[0], w9[1], w9[2], w9[3], w9[4] * 0.5,
                        w9[5], w9[6], w9[7], w9[4] * 0.5, w9[8]])
        d[f"w_{name}"] = _f8(w10)
        csb(name, inp[f"{name}_s"] / WS, inp[f"{name}_b"], 1)
    w = inp["cvm2_w"][:, :, 0, 0].reshape(C, C, 4)  # [co, c, ab]
    d["w_cvm2"] = _bf(np.transpose(w, (2, 1, 0)))  # [ab, ci, co]
    csb("cvm2", inp["cvm2_s"], inp["cvm2_b"], 1)
    w = inp["cvm3_w"][:, :, 0, 0].reshape(C, C, 16)
    d["w_cvm3"] = _bf(np.transpose(w, (2, 1, 0)))
    csb("cvm3", inp["cvm3_s"], inp["cvm3_b"], 1)
    w = inp["cv2_w"][:, :, 0, 0]  # [256, 640]
    d["w_cv2"] = _bf(w.T.reshape(5, C, 2 * C))
    csb("cv2", inp["cv2_s"], inp["cv2_b"], 2)
    d["w_dn"] = _bf(inp["u3_down_w"][:, :, 0, 0].T)  # [128 ci, 32]
    sball[0:32, 24] = inp["u3_down_b"].astype(np.float32)
    w = inp["u3_enc_w"]  # [64, 32, 2, 2]
    d["w_en"] = _bf(np.transpose(w, (2, 3, 1, 0)).reshape(4, 32, 64))
    sball[0:64, 25] = inp["u3_enc_b"].astype(np.float32)
    d["sball"] = sball
    i_idx = np.arange(64)
    d["s64"] = (i_idx[:, None] % 16 == i_idx[None, :] % 16).astype(np.float32)
    d["ident"] = _bf(np.eye(C, dtype=np.float32))
    return d


_NC_CACHE = {}
_TRACE = False  # test.py can flip this to capture an NTFF profile
_LAST_RESULT = None


def get_nc():
    if "nc" not in _NC_CACHE:
        nc = build_nc()
        nc.finalize()  # Bacc: run wait-splitting/reg-alloc passes before lowering
        _NC_CACHE["nc"] = nc
    return _NC_CACHE["nc"]


def make_in_maps(inputs):
    base = prep_base_inputs(inputs)
    x = inputs["x"]  # [8, 256, 128, 128] f32
    xb = _bf(x.reshape(N_CORES, 2, C, NPIX))
    return [dict(base, x=np.ascontiguousarray(xb[i])) for i in range(N_CORES)]


def kernel(**inputs):
    global _LAST_RESULT
    from concourse.bass_utils import run_bass_kernel_spmd

    nc = get_nc()
    in_maps = make_in_maps(inputs)
    res = run_bass_kernel_spmd(
        nc, in_maps, core_ids=list(range(N_CORES)), trace=_TRACE
    )
    _LAST_RESULT = res
    outs = [res.results[i]["out"].reshape(2 * C, H, W) for i in range(N_CORES)]
    return np.stack(outs).astype(np.float32)



# revision 56
# speedup vs baseline: 1.1687x; 1.0107x over previous
"""Trainium2 Bass kernel for the dense CNN (CSP block with CARAFE upsamplers).

Strategy: pure data parallelism — 8 samples over 8 NeuronCores, one sample per
core, full forward pass per core. 285424 ns cost-model exec (vs 534567 ns for
the bf16 version), rel err ~0.011 vs the f32 reference (gate 2e-2).

Pipeline per core:
  cv1 (1x1, 256->256, bf16) -> split y / y0
  bneck1..3 = pairs of 3x3 CBS convs in fp8e4m3 DoubleRow   [y1, y2, y3]
  cvm2/cvm3 (1x1 over pixel_unshuffle, bf16), carafe2 == NN upsample,
  carafe4 (k=2,up=4) softmax reassembly on DVE, cv2 (1x1 640->256, bf16)

fp8 design (the big lever vs the bf16 baseline):
- The six 3x3 convs run with fp8e4m3 weights (pre-scaled x64, exactly unfolded
  via the BN scale) and fp8 input maps; PSUM accumulation stays f32. Each conv
  is FIVE DoubleRow pair-matmuls (0.5 cyc/row): the center tap is split into
  two exactly-halved fp8 copies (slots 4 and 8 of a 10-slot layout) so no
  full-rate single tap remains. 1280 cycles per 512-px tile vs 4608 bf16.
- Accuracy boundary (measured): anything cv2 reads directly must stay bf16.
  fp8-rounding y0 into cv2 costs +1.6e-2 rel err (fails); fp8 x/cv1 costs
  +2.4e-2 (fails). So y/y0/y1/y2/y3-as-cv2-input, cvm2/cvm3/cv2 are bf16, and
  y0 keeps a separate fp8 shadow map (DVE copies chasing cv1's stages) for
  bneck1 only. Conv-internal maps (t1/t2/t3, y2pre, y3pre) are fp8.
- Output DMA is bf16 (host casts back to f32): halves the 16.8MB out traffic.

Scheduling (the rest of the win):
- All activations are 1024-px (two 512-col PSUM banks per group, one wide
  ScalarE activation) to amortize the ~370ns SBUF/PSUM access latency.
- Startup SP queue carries only sball + cv1 weights + the x stream; all
  phase-F/G constants load after phase A's emission. Phase A spills
  (y_sp/y0/y1) stream behind the x loads.
- acc4 (cv2 partial over y,y0,y1,y2) fills the PE during the CARAFE DVE
  window; its last 2 stages stay in SBUF (ring-buffer reuse of the ct pool),
  skipping 8.4MB of the DRAM bounce. Spills stream per-1024px on the SP queue
  with yseg/ct prefetched one stage ahead.
- In phase G the acc4 re-injection is a DVE add (PSUM + SBUF -> SBUF) instead
  of an identity matmul, freeing 13.7us of PE; m3b tiles 0-6 are emitted
  between m3a's two halves so the scalar silu pipeline starts ~10us early.
- carafe2 adds are emitted per 16-z2-row group so m2a starts right after the
  first group instead of after the whole half.

HW pitfalls found on real silicon (these pass CoreSim + birsim but hang or
NaN on device — do NOT reintroduce):
- GPSIMD (Pool) tensor ops cannot touch PSUM (BIR verifier rejects).
- Splitting the CARAFE kernel-prediction chain (cvm3/down/enc/softmax) into
  row-halves and interleaving it inside m2b's conv stream corrupts the
  softmax weights nondeterministically (wn values > 1 / inf): keep the
  z3 -> down -> enc -> exp -> tap-sum -> reciprocal -> wn-bounce chain
  monolithic and half-unsplit, exactly as in kt_chain().

Measured dead ends (sim): h-splitting reassembly chunks (+3.7us: op overhead
beats the earlier m3a unblock); Pool-offloading a blend chunk (+0.6us: Pool
is busy issuing wb SWDGE broadcasts, and a Pool-resident chunk head-blocks
later wb issues). Remaining slack: ~11us PE idle before bneck3 (m3a waits on
4 reassembly chunks), ~6us drain, G tail is ScalarE-bound (~50us of silu).
fp8 for cv1/x and further DMA cuts are accuracy-blocked.
"""
import sys

sys.path.insert(0, "/opt/trn_rl_repo")

import numpy as np
import ml_dtypes

import concourse.bass as bass
import concourse.bacc as bacc
import concourse.mybir as mybir
import concourse.tile as tile

F32 = mybir.dt.float32
BF16 = mybir.dt.bfloat16
F8 = mybir.dt.float8e4
AF = mybir.ActivationFunctionType
DR = mybir.MatmulPerfMode.DoubleRow
WS = 64.0  # fp8 3x3-conv weight pre-scale (folded out via the BN scale)

N_CORES = 8
C = 128
H = W = 128
HP = H + 2  # padded row length
NPIX = H * W
IOFF = HP + 1  # offset of interior (1,1) in padded layout
ZP = 33  # z3 logical padded side (rows/cols -1..31)
ZS = 34  # z3 row stride (even, for DVE 2x alignment)
TAPS9 = [(dy, dx) for dy in (-1, 0, 1) for dx in (-1, 0, 1)]


def _ap(t, off, dims):
    """Free-dim AP into tile t (keeps full partition dim)."""
    return bass.AP(tensor=t.tensor, offset=t.offset + off, ap=[list(t.ap[0])] + dims)


def _zero_border(nc, mp, side=HP):
    nc.vector.memset(_ap(mp, 0, [[1, side]]), 0.0)
    nc.vector.memset(_ap(mp, (side - 1) * side, [[1, side]]), 0.0)
    nc.vector.memset(_ap(mp, side, [[side, side - 2]]), 0.0)
    nc.vector.memset(_ap(mp, side + side - 1, [[side, side - 2]]), 0.0)


def build_nc():
    nc = bacc.Bacc(None)

    # ---- I/O ----
    x_d = nc.dram_tensor("x", [2, C, NPIX], BF16, kind="ExternalInput")
    w_m = {}
    for name in ("m1a", "m1b", "m2a", "m2b", "m3a", "m3b"):
        w_m[name] = nc.dram_tensor(f"w_{name}", [10, C, C], F8, kind="ExternalInput")
    w_cv1 = nc.dram_tensor("w_cv1", [2, C, 2 * C], BF16, kind="ExternalInput")
    w_cvm2 = nc.dram_tensor("w_cvm2", [4, C, C], BF16, kind="ExternalInput")
    w_cvm3 = nc.dram_tensor("w_cvm3", [16, C, C], BF16, kind="ExternalInput")
    w_cv2 = nc.dram_tensor("w_cv2", [5, C, 2 * C], BF16, kind="ExternalInput")
    w_dn = nc.dram_tensor("w_dn", [C, 32], BF16, kind="ExternalInput")
    w_en = nc.dram_tensor("w_en", [4, 32, 64], BF16, kind="ExternalInput")
    s64_d = nc.dram_tensor("s64", [64, 64], F32, kind="ExternalInput")
    sball_d = nc.dram_tensor("sball", [C, 26], F32, kind="ExternalInput")
    out_d = nc.dram_tensor("out", [2, C, NPIX], BF16, kind="ExternalOutput")

    # ---- DRAM scratch ----
    y_sp = nc.dram_tensor("y_sp", [C, NPIX], BF16)  # px-linear
    pad_sp = {k: nc.dram_tensor(f"{k}_sp", [C, HP * HP], BF16) for k in ("y0", "y1")}
    wn_sp = nc.dram_tensor("wn_sp", [64 * 1024], BF16)
    acc4_sp = nc.dram_tensor("acc4_sp", [2, C, NPIX], BF16)  # cv2 partial (y,y0,y1,y2)

    with tile.TileContext(nc) as tc:
        import contextlib

        est = contextlib.ExitStack()
        with est:
            consts = est.enter_context(tc.tile_pool(name="consts", bufs=1))
            maps = est.enter_context(tc.tile_pool(name="maps", bufs=2))
            psum = est.enter_context(tc.tile_pool(name="psum", bufs=2, space="PSUM"))
            # cv2-partial stream tiles (y0,y1 padded row-blocks); opened at top
            # level so stages 0-3 can prefetch during bneck2's PE-bound window
            phFpre = est.enter_context(tc.tile_pool(name="phFpre", bufs=3))
            phFy = est.enter_context(tc.tile_pool(name="phFy", bufs=2))
            cts = {}

            def load_ct(st):
                ct = phFpre.tile([C, 2 * 2080], BF16, tag="ct4", name="ct4")
                poff = (st * 16 + 1) * HP
                nc.sync.dma_start(out=ct[:, 0:2080],
                                  in_=pad_sp["y0"][:, poff:poff + 2080])
                nc.sync.dma_start(out=ct[:, 2080:2 * 2080],
                                  in_=pad_sp["y1"][:, poff:poff + 2080])
                cts[st] = ct

            # ---- constants (small, plus cv2 weights + identity: persistent) ----
            sball = consts.tile([C, 26], F32, tag="sball", name="sball")
            nc.sync.dma_start(out=sball, in_=sball_d[:, :])
            sb = {}
            off = 0
            for name, w_ in (("cv1", 4), ("cv2", 4), ("m1a", 2), ("m1b", 2),
                             ("m2a", 2), ("m2b", 2), ("m3a", 2), ("m3b", 2),
                             ("cvm2", 2), ("cvm3", 2)):
                sb[name] = sball[:, off:off + w_]
                off += w_
            sb["dn_b"] = sball[0:32, 24:25]
            sb["en_b"] = sball[0:64, 25:26]

            wsb = {}

            def load_w(pool, name, src, shape, dt=BF16):
                t = pool.tile(shape, dt, tag=f"w_{name}", name=f"w_{name}")
                nc.sync.dma_start(out=t, in_=src[:, :, :].rearrange("t p n -> p t n"))
                wsb[name] = t

            # 10 tap slots: [t0..t3, t4/2, t5, t6, t7, t4/2, t8] -- the
            # center tap is split into two exactly-halved fp8 copies so all
            # five pairs run in DoubleRow mode (no full-rate single tap).
            TAPS10 = [(-1, -1), (-1, 0), (-1, 1), (0, -1), (0, 0),
                      (0, 1), (1, -1), (1, 0), (0, 0), (1, 1)]

            def taps9_fp8(ps, wt, src, off0):
                """9-tap accumulation as 5 fp8 DoubleRow pair-matmuls;
                off0 = offset of the (dy=-1,dx=-1) tap window."""
                for p in range(5):
                    dy0, dx0 = TAPS10[2 * p]
                    dy1, dx1 = TAPS10[2 * p + 1]
                    delta = (dy1 - dy0) * HP + (dx1 - dx0)
                    rhs = bass.AP(
                        tensor=src.tensor,
                        offset=src.offset + off0 + (dy0 + 1) * HP + (dx0 + 1),
                        ap=[list(src.ap[0]), [delta, 2], [HP, 4], [1, W]],
                    )
                    nc.tensor.matmul(ps, wt[:, 2 * p:2 * p + 2, :], rhs,
                                     start=(p == 0), stop=(p == 4), perf_mode=DR)

            def conv3x3_split(srcA, srcB, dst, wname, i0=0, i1=16,
                                  border=True):
                """CBS 3x3 whose input lives in two row-half maps (66 rows each:
                A = hi-res rows -1..64, B = rows 63..128)."""
                wt = wsb[wname]
                s_ap, b_ap = sb[wname][:, 0:1], sb[wname][:, 1:2]
                if border:
                    _zero_border(nc, dst)
                for i in range(i0, i1):
                    srcm = srcA if i < 8 else srcB
                    ii = i if i < 8 else i - 8
                    ps = psum.tile([C, 1024], F32, tag="ps", name="ps")
                    for h in range(2):
                        taps9_fp8(ps[:, h * 512:(h + 1) * 512], wt, srcm,
                                  (8 * ii + 4 * h) * HP)
                    nc.scalar.activation(
                        _ap(dst, IOFF + 8 * i * HP, [[HP, 8], [1, W]]),
                        ps[:, :].rearrange("p (r w) -> p r w", r=8),
                        AF.Silu, bias=b_ap, scale=s_ap,
                    )

            def conv3x3(src, dst, wname, i0=0, i1=16, border=True):
                """CBS 3x3: src padded map -> dst padded map (interior)."""
                wt = wsb[wname]
                s_ap, b_ap = sb[wname][:, 0:1], sb[wname][:, 1:2]
                if border:
                    _zero_border(nc, dst)
                for i in range(i0, i1):
                    ps = psum.tile([C, 1024], F32, tag="ps", name="ps")
                    for h in range(2):
                        taps9_fp8(ps[:, h * 512:(h + 1) * 512], wt, src,
                                  (8 * i + 4 * h) * HP)
                    nc.scalar.activation(
                        _ap(dst, IOFF + 8 * i * HP, [[HP, 8], [1, W]]),
                        ps[:, :].rearrange("p (r w) -> p r w", r=8),
                        AF.Silu, bias=b_ap, scale=s_ap,
                    )

            with tc.tile_pool(name="wearly", bufs=1) as wearly:
                load_w(consts, "cv1", w_cv1, [C, 2, 2 * C])

                # ================= Phase A: cv1 =================
                y0p = maps.tile([C, HP * HP], BF16, tag="m")
                _zero_border(nc, y0p)
                # fp8 shadow of y0 for bneck1 (cv2/spill keep the bf16 map);
                # DVE copies chase cv1's stages in the otherwise-idle window
                y0p8 = wearly.tile([C, HP * HP], F8, tag="y08")
                _zero_border(nc, y0p8)
                with tc.tile_pool(name="phA", bufs=3) as phA, \
                     tc.tile_pool(name="phAy", bufs=3) as phAy:
                    for st in range(8):  # stages of 2048 px (16 rows)
                        xt = phA.tile([C, 2, 2048], BF16, tag="xt", name="xt")
                        for kt in range(2):
                            nc.sync.dma_start(
                                out=xt[:, kt, :], in_=x_d[kt, :, st * 2048:(st + 1) * 2048]
                            )
                        yt = phAy.tile([C, 2048], BF16, tag="yt", name="yt")
                        for jp in range(2):
                            row0 = 16 * st + 8 * jp
                            psy = psum.tile([C, 1024], F32, tag="ps", name="ps")
                            psy0 = psum.tile([C, 1024], F32, tag="ps", name="ps")
                            for h in range(2):
                                xs = xt[:, :, (2 * jp + h) * 512:(2 * jp + h + 1) * 512]
                                for kt in range(2):
                                    nc.tensor.matmul(
                                        psy[:, h * 512:(h + 1) * 512],
                                        wsb["cv1"][:, kt, 0:C], xs[:, kt, :],
                                        start=(kt == 0), stop=(kt == 1),
                                    )
                                for kt in range(2):
                                    nc.tensor.matmul(
                                        psy0[:, h * 512:(h + 1) * 512],
                                        wsb["cv1"][:, kt, C:2 * C], xs[:, kt, :],
                                        start=(kt == 0), stop=(kt == 1),
                                    )
                            nc.scalar.activation(
                                yt[:, jp * 1024:(jp + 1) * 1024], psy, AF.Silu,
                                bias=sb["cv1"][:, 2:3], scale=sb["cv1"][:, 0:1],
                            )
                            nc.scalar.activation(
                                _ap(y0p, IOFF + row0 * HP, [[HP, 8], [1, W]]),
                                psy0[:, :].rearrange("p (r w) -> p r w", r=8),
                                AF.Silu, bias=sb["cv1"][:, 3:4], scale=sb["cv1"][:, 1:2],
                            )
                        nc.sync.dma_start(
                            out=y_sp[:, st * 2048:(st + 1) * 2048], in_=yt
                        )
                        nc.vector.tensor_copy(
                            _ap(y0p8, IOFF + 16 * st * HP, [[HP, 16], [1, W]]),
                            _ap(y0p, IOFF + 16 * st * HP, [[HP, 16], [1, W]]),
                        )
                nc.sync.dma_start(out=pad_sp["y0"][:, :], in_=y0p)

                # phase-F/G constants: loaded after phase A's emission so the
                # startup SP queue is just sball + cv1 weights + the x stream
                s64 = consts.tile([64, 64], F32, tag="s64")
                nc.sync.dma_start(out=s64, in_=s64_d[:, :])
                wdn = consts.tile([C, 32], BF16, tag="wdn")
                nc.sync.dma_start(out=wdn, in_=w_dn[:, :])
                wen = consts.tile([32, 4, 64], BF16, tag="wen")
                nc.sync.dma_start(out=wen, in_=w_en[:, :, :].rearrange("t p n -> p t n"))
                wcv2 = consts.tile([C, 5, 2 * C], BF16, tag="wcv2")
                nc.sync.dma_start(out=wcv2, in_=w_cv2[:, :, :].rearrange("t p n -> p t n"))
                # remaining early weights (loads overlap with phase A/B compute)
                load_w(consts, "m1a", w_m["m1a"], [C, 10, C], dt=F8)
                load_w(consts, "m1b", w_m["m1b"], [C, 10, C], dt=F8)
                load_w(consts, "cvm2", w_cvm2, [C, 4, C])
                load_w(consts, "m2a", w_m["m2a"], [C, 10, C], dt=F8)
                load_w(consts, "m2b", w_m["m2b"], [C, 10, C], dt=F8)

                # ================= Phase B: bneck1 =================
                with tc.tile_pool(name="phB", bufs=1) as phB:
                    t1p = phB.tile([C, HP * HP], F8, tag="aux", name="t1p")
                    conv3x3(y0p8, t1p, "m1a")
                    y1p = maps.tile([C, HP * HP], BF16, tag="m")
                    conv3x3(t1p, y1p, "m1b")
            nc.sync.dma_start(out=pad_sp["y1"][:, :], in_=y1p)
            load_w(consts, "cvm3", w_cvm3, [C, 16, C])
            load_w(consts, "m3a", w_m["m3a"], [C, 10, C], dt=F8)
            load_w(consts, "m3b", w_m["m3b"], [C, 10, C], dt=F8)

            # ================= Phase C/D: cvm2 + carafe2 =================
            for st in range(3):
                load_ct(st)
            y2pp = maps.tile([C, HP * HP], F8, tag="m")
            _zero_border(nc, y2pp)
            for half in range(2):
                z2 = phFy.tile([C, 2048], BF16, tag="yseg", name="z2")
                for ip in range(2):  # 16 z2-rows of 64 px per psum tile
                    ps = psum.tile([C, 1024], F32, tag="ps", name="ps")
                    for h in range(2):
                        i = 2 * ip + h
                        for abi in range(4):
                            a, b = abi // 2, abi % 2
                            rhs = _ap(y1p, ((64 * half + 16 * i) + a + 1) * HP + (b + 1),
                                      [[2 * HP, 8], [2, 64]])
                            nc.tensor.matmul(ps[:, h * 512:(h + 1) * 512],
                                             wsb["cvm2"][:, abi, :], rhs,
                                             start=(abi == 0), stop=(abi == 3))
                    nc.scalar.activation(
                        z2[:, ip * 1024:(ip + 1) * 1024], ps, AF.Silu,
                        bias=sb["cvm2"][:, 1:2], scale=sb["cvm2"][:, 0:1],
                    )
                    # carafe2 == NN 2x upsample + y1, per 16-z2-row group so
                    # m2a's first tiles unblock right after the first group
                    for abi in range(4):
                        a, b = abi // 2, abi % 2
                        pos = [[2 * HP, 16], [2, 64]]
                        roff = (64 * half + 32 * ip + a) * HP + b
                        nc.vector.tensor_tensor(
                            out=_ap(y2pp, IOFF + roff, pos),
                            in0=z2[:, ip * 1024:(ip + 1) * 1024]
                            .rearrange("p (h w) -> p h w", h=16),
                            in1=_ap(y1p, IOFF + roff, pos),
                            op=mybir.AluOpType.add,
                        )

            # ==== Phases E+F: bneck2 + cvm3/carafe4 + cv2 partial ====
            # The CARAFE kernel-prediction chain is interleaved between m2b's
            # two row-halves: region-granular tile deps let cvm3-h0 + the
            # down/enc/softmax chain run on PE/ScalarE/DVE/DMA during m2b's
            # PE-bound second half, so the reassembly DVE block starts almost
            # immediately after m2b (previously ~13us of serial kt latency).
            with tc.tile_pool(name="phFy3", bufs=1) as phFy3:
                with tc.tile_pool(name="phE", bufs=1) as phE, \
                     tc.tile_pool(name="phF", bufs=1) as phF, \
                     tc.tile_pool(name="phFwb", bufs=2) as phFwb, \
                     tc.tile_pool(name="phFtmp", bufs=1) as phFtmp, \
                     tc.tile_pool(name="phFacc", bufs=2) as phFacc, \
                     tc.tile_pool(name="phFac", bufs=3) as phFac, \
                     tc.tile_pool(name="psF", bufs=2, space="PSUM") as psF:
                    t2p = phE.tile([C, HP * HP], F8, tag="aux", name="t2p")
                    conv3x3(y2pp, t2p, "m2a")
                    y2p = maps.tile([C, HP * HP], BF16, tag="m")

                    # ---- F-prep tiles; memsets run early on the idle DVE ----
                    z3a = phF.tile([C, ZS * ZP], BF16, tag="z3a", name="z3a")
                    nc.vector.memset(z3a, 0.0)
                    z3b = phF.tile([C, ZS * ZP], BF16, tag="z3b", name="z3b")
                    nc.vector.memset(z3b, 0.0)
                    kt1p = phF.tile([32, ZP * ZP], BF16, tag="kt1p", name="kt1p")
                    _zero_border(nc, kt1p, side=ZP)
                    e_sb = phF.tile([64, 1024], F32, tag="e", name="e_sb")
                    rden = phF.tile([64, 1024], F32, tag="rden", name="rden")
                    wnt = phF.tile([64, 1024], BF16, tag="wnt", name="wnt")
                    y3A = phFy3.tile([C, 66 * HP], F8, tag="y3A", name="y3A")
                    y3B = phFy3.tile([C, 66 * HP], F8, tag="y3B", name="y3B")
                    nc.vector.memset(_ap(y3A, 0, [[1, HP]]), 0.0)
                    nc.vector.memset(_ap(y3B, 65 * HP, [[1, HP]]), 0.0)
                    for mm in (y3A, y3B):
                        nc.vector.memset(_ap(mm, 0, [[HP, 66]]), 0.0)
                        nc.vector.memset(_ap(mm, HP - 1, [[HP, 66]]), 0.0)
                    wn_flat = wn_sp[:]

                    def kt_chain():
                        """cvm3 + CARAFE kernel prediction, baseline form."""
                        for i in range(2):  # 16 z3-rows of 32 px per tile
                            psz = psum.tile([C, 512], F32, tag="ps", name="psz")
                            for abi in range(16):
                                a, b = abi // 4, abi % 4
                                rhs = _ap(y2p, (64 * i + a + 1) * HP + (b + 1),
                                          [[4 * HP, 16], [4, 32]])
                                nc.tensor.matmul(psz, wsb["cvm3"][:, abi, :], rhs,
                                                 start=(abi == 0), stop=(abi == 15))
                            nc.scalar.activation(
                                _ap(z3a, (16 * i + 1) * ZS + 1, [[ZS, 16], [1, 32]]),
                                psz[:, :].rearrange("p (r w) -> p r w", r=16),
                                AF.Silu, bias=sb["cvm3"][:, 1:2],
                                scale=sb["cvm3"][:, 0:1],
                            )
                        for i in range(2):
                            ps1 = psF.tile([32, 512], F32, tag="psk", name="psk")
                            nc.tensor.matmul(
                                ps1, wdn,
                                _ap(z3a, (16 * i + 1) * ZS + 1, [[ZS, 16], [1, 32]]),
                                start=True, stop=True,
                            )
                            nc.scalar.activation(
                                _ap(kt1p, (16 * i + 1) * ZP + 1, [[ZP, 16], [1, 32]]),
                                ps1[:, :].rearrange("p (r w) -> p r w", r=16),
                                AF.Identity, bias=sb["dn_b"][:, 0:1],
                            )
                        for i in range(2):
                            ps2 = psF.tile([64, 512], F32, tag="psk", name="psk")
                            for t in range(4):
                                di, dj = t // 2, t % 2
                                rhs = bass.AP(
                                    tensor=kt1p.tensor,
                                    offset=kt1p.offset + (16 * i + di) * ZP + dj,
                                    ap=[list(kt1p.ap[0]), [ZP, 16], [1, 32]],
                                )
                                nc.tensor.matmul(ps2, wen[:, t, :], rhs,
                                                 start=(t == 0), stop=(t == 3))
                            nc.scalar.activation(
                                e_sb[:, i * 512:(i + 1) * 512],
                                ps2[:, :].rearrange("p (r w) -> p r w", r=16),
                                AF.Exp, bias=sb["en_b"][:, 0:1],
                            )
                        sm = psF.tile([64, 1024], F32, tag="psk", name="psk")
                        for i in range(2):
                            nc.tensor.matmul(sm[:, i * 512:(i + 1) * 512], s64,
                                             e_sb[:, i * 512:(i + 1) * 512],
                                             start=True, stop=True)
                        nc.vector.reciprocal(rden, sm)
                        nc.vector.tensor_tensor(out=wnt, in0=e_sb, in1=rden,
                                                op=mybir.AluOpType.mult)
                        nc.sync.dma_start(
                            out=wn_sp[:].rearrange("(p f) -> p f", p=64), in_=wnt)

                    # ---- m2b with the kt chain slotted between the halves ----
                    conv3x3(t2p, y2p, "m2b")
                    load_ct(3)
                    kt_chain()

                    # column-shifted copy (even tap starts) + low-res tap diffs
                    nc.vector.tensor_copy(
                        _ap(z3b, 0, [[ZS, ZP], [1, ZP - 1]]),
                        _ap(z3a, 1, [[ZS, ZP], [1, ZP - 1]]),
                    )
                    dts = {}
                    for t in (1, 2, 3):
                        ti, tj = t // 2, t % 2
                        dt = phF.tile([C, 1024], BF16, tag=f"d{t}", name=f"d{t}")
                        nc.vector.tensor_tensor(
                            out=dt,
                            in0=_ap(z3b if tj else z3a, ti * ZS, [[ZS, 32], [1, 32]]),
                            in1=_ap(z3a, 0, [[ZS, 32], [1, 32]]),
                            op=mybir.AluOpType.subtract,
                        )
                        dts[t] = dt

                    # ---- cv2 partial: acc4 = w_y.y + w_y0.y0 + w_y1.y1 + w_y2.y2
                    # (PE bulk that runs concurrently with the DVE reassembly)
                    kept = {}
                    ysegs = {}

                    def load_yseg(st):
                        yseg = phFy.tile([C, 2048], BF16, tag="yseg", name="yseg")
                        nc.sync.dma_start(out=yseg,
                                          in_=y_sp[:, st * 2048:(st + 1) * 2048])
                        ysegs[st] = yseg

                    load_yseg(0)
                    for st in range(8):
                        if st >= 4:
                            load_ct(st)
                        if st < 7:
                            load_yseg(st + 1)
                        ct = cts.pop(st)
                        yseg = ysegs.pop(st)
                        if st >= 6:
                            # last 2 stages stay in SBUF (phFpre ring is free by
                            # now); phase G reads them directly, skipping DRAM
                            atk = phFpre.tile([C, 2, 2048], BF16, tag="ct4",
                                              name="atk")
                            kept[st] = atk
                        for jp in range(2):
                            for co in range(2):
                                ps = psum.tile([C, 1024], F32, tag="ps", name="ps")
                                for h in range(2):
                                    j = 2 * jp + h
                                    pss = ps[:, h * 512:(h + 1) * 512]
                                    nc.tensor.matmul(
                                        pss, wcv2[:, 0, co * C:(co + 1) * C],
                                        yseg[:, j * 512:(j + 1) * 512],
                                        start=True, stop=False,
                                    )
                                    for ki, soff in ((1, 0), (2, 2080)):
                                        nc.tensor.matmul(
                                            pss, wcv2[:, ki, co * C:(co + 1) * C],
                                            _ap(ct, soff + 4 * j * HP + 1,
                                                [[HP, 4], [1, W]]),
                                            start=False, stop=False,
                                        )
                                    nc.tensor.matmul(
                                        pss, wcv2[:, 3, co * C:(co + 1) * C],
                                        _ap(y2p, IOFF + (16 * st + 4 * j) * HP,
                                            [[HP, 4], [1, W]]),
                                        start=False, stop=True,
                                    )
                                if st >= 6:
                                    nc.scalar.copy(
                                        kept[st][:, co, jp * 1024:(jp + 1) * 1024],
                                        ps)
                                else:
                                    att = phFac.tile([C, 1024], BF16, tag="at4",
                                                     name="att")
                                    nc.scalar.copy(att, ps)
                                    nc.sync.dma_start(
                                        out=acc4_sp[co, :,
                                                    st * 2048 + jp * 1024:
                                                    st * 2048 + (jp + 1) * 1024],
                                        in_=att)

                    # ---- reassembly: out_s = z00 + sum_{t!=00} w_t*(z_t - z00),
                    # half-0 chunks first (their weights bounced during m2b);
                    # half-1's normalization interleaves behind two chunks.
                    for half, q in ((0, 0), (0, 1), (0, 2), (0, 3),
                                    (1, 0), (1, 1), (1, 2), (1, 3)):
                        hoff = 16 * half
                        accq = phFacc.tile([C, 2048], BF16, tag="accq", name="accq")
                        dst = accq[:, :]
                        for t in (1, 2, 3):
                            wb = phFwb.tile([C, 2048], BF16, tag="wb", name="wb")
                            src2 = bass.AP(
                                tensor=wn_flat.tensor,
                                offset=wn_flat.offset + t * 16384 + q * 4096
                                + half * 512,
                                ap=[[0, C], [1024, 4], [1, 512]],
                            )
                            nc.gpsimd.dma_start(out=wb, in_=src2)
                            dread = _ap(dts[t], half * 512,
                                        [[0, 4], [32, 16], [1, 32]])
                            if t == 1:
                                nc.vector.tensor_tensor(out=dst, in0=wb, in1=dread,
                                                        op=mybir.AluOpType.mult)
                            else:
                                tmp = phFtmp.tile([C, 2048], BF16, tag="tmp",
                                                  name="tmp")
                                nc.vector.tensor_tensor(out=tmp, in0=wb, in1=dread,
                                                        op=mybir.AluOpType.mult)
                                nc.vector.tensor_tensor(out=dst, in0=dst, in1=tmp,
                                                        op=mybir.AluOpType.add)
                        nc.vector.tensor_tensor(
                            out=dst, in0=dst,
                            in1=_ap(z3a, (hoff) * ZS, [[0, 4], [ZS, 16], [1, 32]]),
                            op=mybir.AluOpType.add,
                        )
                        # + y2 residual, rows R = 4h+q for h in this half
                        ymap = y3A if half == 0 else y3B
                        pos = [[1, 4], [4 * HP, 16], [4, 32]]
                        nc.vector.tensor_tensor(
                            out=_ap(ymap, (q + 1) * HP + 1, pos),
                            in0=accq[:, :],
                            in1=_ap(y2p, IOFF + (64 * half + q) * HP, pos),
                            op=mybir.AluOpType.add,
                        )
                        # boundary rows shared by both halves
                        if half == 0 and q == 3:  # R=63 -> B row 0
                            nc.vector.tensor_tensor(
                                out=_ap(y3B, 1, [[1, 4], [4, 32]]),
                                in0=_ap(accq, 15 * 32, [[512, 4], [1, 32]]),
                                in1=_ap(y2p, IOFF + 63 * HP, [[1, 4], [4, 32]]),
                                op=mybir.AluOpType.add,
                            )
                        if half == 1 and q == 0:  # R=64 -> A row 65
                            nc.vector.tensor_tensor(
                                out=_ap(y3A, 65 * HP + 1, [[1, 4], [4, 32]]),
                                in0=_ap(accq, 0, [[512, 4], [1, 32]]),
                                in1=_ap(y2p, IOFF + 64 * HP, [[1, 4], [4, 32]]),
                                op=mybir.AluOpType.add,
                            )

                # ===== Phase G: bneck3 + cv2 final, chained per 4-row tile =====
                with tc.tile_pool(name="phGy", bufs=3) as phGy, \
                     tc.tile_pool(name="phGc", bufs=2) as phGc, \
                     tc.tile_pool(name="phGa", bufs=3) as phGa, \
                     tc.tile_pool(name="phGo", bufs=2) as phGo, \
                     tc.tile_pool(name="psG", bufs=2, space="PSUM") as psG:
                    t3p = maps.tile([C, HP * HP], F8, tag="m", name="t3p")

                    # m3b produces y3 in 8-row px-linear tiles; cv2 consumes each
                    # immediately (y3 is only ever read by cv2's 1x1 conv). The
                    # acc4 partial is re-injected on the DVE (idle here), not via
                    # an identity matmul, so the PE only runs m3b + the y3 k-tile.
                    # m3b tiles 0-6 (which only read t3p rows <= 56) are emitted
                    # between m3a's halves so the scalar pipeline starts early.
                    wt = wsb["m3b"]
                    s3, b3 = sb["m3b"][:, 0:1], sb["m3b"][:, 1:2]
                    ca = None

                    def m3b_tile(i):
                        nonlocal ca
                        st, jj = i // 2, i % 2
                        if jj == 0:
                            if st in kept:
                                ca = kept[st]
                            else:
                                ca = phGc.tile([C, 2, 2048], BF16, tag="ca",
                                               name="ca")
                                nc.sync.dma_start(
                                    out=ca,
                                    in_=acc4_sp[:, :, st * 2048:(st + 1) * 2048]
                                    .rearrange("k p f -> p k f"))
                        ot = phGo.tile([C, 2, 1024], BF16, tag="ot", name="ot")
                        ps = psum.tile([C, 1024], F32, tag="ps", name="ps")
                        for h in range(2):
                            taps9_fp8(ps[:, h * 512:(h + 1) * 512], wt, t3p,
                                      (8 * i + 4 * h) * HP)
                        y3t = phGy.tile([C, 1024], BF16, tag="y3t", name="y3t")
                        nc.scalar.activation(y3t, ps, AF.Silu, bias=b3, scale=s3)
                        for co in range(2):
                            ps2 = psG.tile([C, 1024], F32, tag="ps2", name="ps2")
                            for h in range(2):
                                nc.tensor.matmul(
                                    ps2[:, h * 512:(h + 1) * 512],
                                    wcv2[:, 4, co * C:(co + 1) * C],
                                    y3t[:, h * 512:(h + 1) * 512],
                                    start=True, stop=True,
                                )
                            sm_t = phGa.tile([C, 1024], F32, tag="sm", name="sm_t")
                            nc.vector.tensor_tensor(
                                out=sm_t, in0=ps2,
                                in1=ca[:, co, jj * 1024:(jj + 1) * 1024],
                                op=mybir.AluOpType.add,
                            )
                            nc.scalar.activation(
                                ot[:, co, :], sm_t, AF.Silu,
                                bias=sb["cv2"][:, 2 + co:3 + co],
                                scale=sb["cv2"][:, co:co + 1],
                            )
                        base = st * 2048 + jj * 1024
                        for co in range(2):
                            nc.sync.dma_start(
                                out=out_d[co, :, base:base + 1024],
                                in_=ot[:, co, :],
                            )

                    conv3x3_split(y3A, y3B, t3p, "m3a", i0=0, i1=8)
                    for i in range(7):
                        m3b_tile(i)
                    conv3x3_split(y3A, y3B, t3p, "m3a", i0=8, i1=16, border=False)
                    for i in range(7, 16):
                        m3b_tile(i)
    return nc


def _bf(a):
    return np.ascontiguousarray(a.astype(ml_dtypes.bfloat16))


def _f8(a):
    assert np.abs(a).max() < 240.0, "fp8e4m3 overflow in weight prep"
    return np.ascontiguousarray(a.astype(ml_dtypes.float8_e4m3))


def prep_base_inputs(inp):
    """Host-side weight rearrangement -> the flat in_map (minus x)."""
    d = {}

    sball = np.zeros((C, 26), np.float32)
    sb_off = {"cv1": 0, "cv2": 4, "m1a": 8, "m1b": 10, "m2a": 12, "m2b": 14,
              "m3a": 16, "m3b": 18, "cvm2": 20, "cvm3": 22}

    def csb(pre, s, b, ntile):
        # scale/bias packed: cols [o:o+ntile]=scale, [o+ntile:o+2*ntile]=bias
        o = sb_off[pre]
        for i in range(ntile):
            sball[:, o + i] = s[i * C:(i + 1) * C]
            sball[:, o + ntile + i] = b[i * C:(i + 1) * C]

    # cv1: w [256, 256, 1, 1] -> [2 (ci tile), 128, 256 co]
    w = inp["cv1_w"][:, :, 0, 0]  # [co, ci]
    d["w_cv1"] = _bf(w.T.reshape(2, C, 2 * C))
    csb("cv1", inp["cv1_s"], inp["cv1_b"], 2)
    for name in ("m1a", "m1b", "m2a", "m2b", "m3a", "m3b"):
        w = inp[f"{name}_w"]  # [co, ci, 3, 3]
        w9 = np.transpose(w, (2, 3, 1, 0)).reshape(9, C, C) * WS
        # 10 slots: center tap exactly halved into slots 4 and 8 so all five
        # DoubleRow pairs are well-formed (fp8 halving is exact)
        w10 = np.stack([w9[0], w9[1], w9[2], w9[3], w9[4] * 0.5,
                        w9[5], w9[6], w9[7], w9[4] * 0.5, w9[8]])
        d[f"w_{name}"] = _f8(w10)
        csb(name, inp[f"{name}_s"] / WS, inp[f"{name}_b"], 1)
    w = inp["cvm2_w"][:, :, 0, 0].reshape(C, C, 4)  # [co, c, ab]
    d["w_cvm2"] = _bf(np.transpose(w, (2, 1, 0)))  # [ab, ci, co]
    csb("cvm2", inp["cvm2_s"], inp["cvm2_b"], 1)
    w = inp["cvm3_w"][:, :, 0, 0].reshape(C, C, 16)
    d["w_cvm3"] = _bf(np.transpose(w, (2, 1, 0)))
    csb("cvm3", inp["cvm3_s"], inp["cvm3_b"], 1)
    w = inp["cv2_w"][:, :, 0, 0]  # [256, 640]
    d["w_cv2"] = _bf(w.T.reshape(5, C, 2 * C))
    csb("cv2", inp["cv2_s"], inp["cv2_b"], 2)
    d["w_dn"] = _bf(inp["u3_down_w"][:, :, 0, 0].T)  # [128 ci, 32]
    sball[0:32, 24] = inp["u3_down_b"].astype(np.float32)
    w = inp["u3_enc_w"]  # [64, 32, 2, 2]
    d["w_en"] = _bf(np.transpose(w, (2, 3, 1, 0)).reshape(4, 32, 64))
    sball[0:64, 25] = inp["u3_enc_b"].astype(np.float32)
    d["sball"] = sball
    i_idx = np.arange(64)
    d["s64"] = (i_idx[:, None] % 16 == i_idx[None, :] % 16).astype(np.float32)
    return d


_NC_CACHE = {}
_TRACE = False  # test.py can flip this to capture an NTFF profile
_LAST_RESULT = None


def get_nc():
    if "nc" not in _NC_CACHE:
        nc = build_nc()
        nc.finalize()  # Bacc: run wait-splitting/reg-alloc passes before lowering
        _NC_CACHE["nc"] = nc
    return _NC_CACHE["nc"]


def make_in_maps(inputs):
    base = prep_base_inputs(inputs)
    x = inputs["x"]  # [8, 256, 128, 128] f32
    xb = _bf(x.reshape(N_CORES, 2, C, NPIX))
    return [dict(base, x=np.ascontiguousarray(xb[i])) for i in range(N_CORES)]


def kernel(**inputs):
    global _LAST_RESULT
    from concourse.bass_utils import run_bass_kernel_spmd

    nc = get_nc()
    in_maps = make_in_maps(inputs)
    res = run_bass_kernel_spmd(
        nc, in_maps, core_ids=list(range(N_CORES)), trace=_TRACE
    )
    _LAST_RESULT = res
    outs = [res.results[i]["out"].reshape(2 * C, H, W) for i in range(N_CORES)]
    return np.stack(outs).astype(np.float32)

